# revision 14
# baseline (speedup 1.0000x reference)
"""Trainium2 Bass kernel for nn_NeuralODE (RK4 satellite propagation, M=10000 steps).

Self-contained: takes FULL inputs (as produced by the problem's setup_inputs),
runs the sequential RK4 integration on NeuronCore 0 (replicated across cores
0-7 via SPMD), returns the full (M, 6) trajectory.

Design notes:
  - state (6,1) on partitions 0-5. MLP (6->32->32->3 tanh) on TensorE+ScalarE.
  - gravity J2..J5 evaluated via a factored form:
      g = r3inv * ( pos * S + e_z * R )
      S = -MU + u*D2(m) + u^2*D4(m) + (z*u^2)*D3(m) + (z*u^3)*D5(m)
      (m = z^2*u, u = 1/r^2; D_k linear in m -- the dropped m^2 terms
       contribute < 1e-9 relative)
    computed as a 5-lane tensor_tensor_reduce with per-partition (x/y/z)
    coefficient tables; R folded into two extra lanes of the z row.
  - 1/r via quadratic polynomial init (fitted on r^2 in [3.9e7, 5.1e7]) plus
    two Newton iterations (validated to sit within the f32 reference's own
    rounding envelope).
  - RK4 stage states & next-state built as tiny PE matmul accumulations:
      s2 = [p0 + (h/2)v0 ; v0 + (h/2)a1]
      s3 = [p2 + (h^2/4)a1 ; v0 + (h/2)a2]
      s4 = [p0 + h v0 + (h^2/2)a2 ; v0 + h a3]
      s' = [p0 + h v0 + (h^2/6)(a1+a2+a3) ; v0 + (h/6)(a1+2a2+2a3+a4)]
  - all dt-dependent constants are folded on the host into input tensors, so
    one compiled NEFF serves any dt.
"""
import os
import numpy as np
from contextlib import ExitStack

MU = 398600.4418
RE = 6378.137
J2 = 1.08262668e-3
J3 = -2.53265648e-6
J4 = -1.61962159e-6
J5 = -2.27296082e-7
R_REF = 7000.0
V_REF = 7.5
A_REF = V_REF * V_REF / R_REF

F32 = np.float32


# ----------------------------------------------------------------------------
# host-side constant folding
# ----------------------------------------------------------------------------

def _fit_rsqrt_quad(lo=3.9e7, hi=5.1e7):
    """least-squares quadratic fit of x^{-1/2} on [lo, hi] (Chebyshev nodes)."""
    xs = (np.cos(np.pi * (np.arange(4000) + 0.5) / 4000) + 1) / 2 * (hi - lo) + lo
    f = xs ** -0.5
    t = 2 * (xs - lo) / (hi - lo) - 1
    V = np.polynomial.chebyshev.chebvander(t, 2)
    c, *_ = np.linalg.lstsq(V, f, rcond=None)
    pt = np.polynomial.chebyshev.cheb2poly(c)
    a = 2 / (hi - lo)
    b = -1 - 2 * lo / (hi - lo)
    px = np.zeros(3)
    for k, ck in enumerate(pt):
        poly = np.array([1.0])
        for _ in range(k):
            poly = np.convolve(poly, np.array([b, a]))
        px[:len(poly)] += ck * poly
    return px  # [c0, c1, c2]: c0 + c1*x + c2*x^2


_PX = _fit_rsqrt_quad()
C0_RSQ = float(_PX[0])
C1_RSQ = float(_PX[1])
C2_RSQ = float(_PX[2])
C1Q_RSQ = C1_RSQ / C2_RSQ  # v0 = C2*((r2 + C1Q)*r2) + C0


def _gravity_tables():
    """(G1, G0) each (3, 7): D = G1*m + G0 per lane; lanes
    [const(-MU), u*D2, u2*D4, (u*zu)*D3, (u2*zu)*D5, R1(u), R2(u2)]."""
    G0 = np.zeros((3, 7))
    G1 = np.zeros((3, 7))
    G0[:, 0] = -MU
    c2 = J2 * MU * RE**2
    G0[:2, 1] = -1.5 * c2
    G1[:2, 1] = 7.5 * c2
    G0[2, 1] = -4.5 * c2
    G1[2, 1] = 7.5 * c2
    c4 = (15.0 / 8.0) * J4 * MU * RE**4
    G0[:2, 2] = c4
    G1[:2, 2] = -14.0 * c4
    G0[2, 2] = 5.0 * c4
    G1[2, 2] = -(70.0 / 3.0) * c4
    c3 = -2.5 * J3 * MU * RE**3
    G0[:2, 3] = 3.0 * c3
    G1[:2, 3] = -7.0 * c3
    G0[2, 3] = 6.0 * c3
    G1[2, 3] = -7.0 * c3
    c5 = (3.0 / 8.0) * J5 * MU * RE**5
    G0[:2, 4] = 35.0 * c5
    G1[:2, 4] = -210.0 * c5
    G0[2, 4] = 105.0 * c5
    G1[2, 4] = -315.0 * c5
    # R lanes (z row only): R = rho1*u + rho2*u2
    G0[2, 5] = 1.5 * J3 * MU * RE**3
    G0[2, 6] = -1.875 * J5 * MU * RE**5
    return F32(G1), F32(G0)


def _amat(alpha):
    """lhsT for out = [[I, alpha*I],[0, I]] @ s, split into pos/vel halves."""
    A = np.zeros((6, 6))
    A[:3, :3] = np.eye(3)
    A[:3, 3:] = alpha * np.eye(3)
    A[3:, 3:] = np.eye(3)
    Ap = A.copy(); Ap[3:, :] = 0.0           # pos rows only
    Av = A.copy(); Av[:3, :] = 0.0           # vel rows only
    return F32(Ap.T), F32(Av.T), F32(A.T)


def _bmat(alpha, beta):
    """lhsT (3,6) for out(6,1) = [alpha*a ; beta*a], a (3,1)."""
    B = np.zeros((6, 3))
    B[:3, :] = alpha * np.eye(3)
    B[3:, :] = beta * np.eye(3)
    return F32(B.T)


def make_host_inputs(state0, W1, b1, W2, b2, W3, b3, log_scale, dt):
    h = float(dt)
    scale_vec = np.array([1 / R_REF] * 3 + [1 / V_REF] * 3, np.float64)
    W1s = F32(np.float64(W1) * scale_vec[None, :])
    amp = F32(F32(np.exp(F32(log_scale))) * F32(A_REF))
    W3e = F32(np.float64(W3) * np.float64(amp))
    b3e = F32(np.float64(b3) * np.float64(amp))
    G1, G0 = _gravity_tables()
    d = {
        "w1spt": np.ascontiguousarray(W1s[:, 0:3].T),   # (3,32)
        "w1svt": np.ascontiguousarray(W1s[:, 3:6].T),   # (3,32)
        "w2t": np.ascontiguousarray(F32(W2).T),         # (32,32)
        "w3et": np.ascontiguousarray(W3e.T),            # (32,3)
        "b1t": F32(b1).reshape(32, 1),
        "b2t": F32(b2).reshape(32, 1),
        "b3t": b3e.reshape(1, 3),
        "g1c": G1, "g0c": G0,                           # (3,7)
        "c0t": F32([[C0_RSQ]]),                         # (1,1)
        "cmv": F32([[1, 0, 0], [1, 0, 0], [1, 1, 1]]),  # (3,3)
        "ones13": F32([[1, 1, 1]]),                     # (1,3)
        "s0t": F32(state0).reshape(6, 1),
    }
    return d


# ----------------------------------------------------------------------------
# kernel builder
# ----------------------------------------------------------------------------

def build(M=10000, U=8, dt=60.0):
    assert U % 2 == 0 and M % U == 0
    return _build_inner(M, U, U // 2, float(dt))


def _build_inner(M, U, H, h):
    import concourse.bacc as bacc
    import concourse.tile as tile
    from concourse import mybir
    from concourse.bass import ds

    f32 = mybir.dt.float32
    Alu = mybir.AluOpType
    Act = mybir.ActivationFunctionType

    hf = float(F32(h))
    h2 = float(F32(hf / 2))
    h6 = float(F32(hf / 6))
    hh = float(F32(F32(hf) * F32(hf)))
    h24 = float(F32(hh / 4))
    h22 = float(F32(hh / 2))
    h26 = float(F32(hh / 6))

    nc = bacc.Bacc(None, target_bir_lowering=False, debug=False)
    names = {}

    with tile.TileContext(nc) as tc, ExitStack() as ctx:
        dram = ctx.enter_context(tc.tile_pool(name="dram", bufs=1, space="DRAM"))
        sing = ctx.enter_context(tc.tile_pool(name="sing", bufs=1))
        work = ctx.enter_context(tc.tile_pool(name="work", bufs=2))
        psum = ctx.enter_context(tc.tile_pool(name="ps", bufs=2, space="PSUM"))

        shapes = {
            "w1spt": (3, 32), "w1svt": (3, 32), "w2t": (32, 32),
            "w3et": (32, 3), "b1t": (32, 1), "b2t": (32, 1), "b3t": (1, 3),
            "g1c": (3, 7), "g0c": (3, 7), "c0t": (1, 1), "cmv": (3, 3),
            "ones13": (1, 3), "s0t": (6, 1),
        }
        dtiles = {}
        sb = {}
        for nm, shp in shapes.items():
            dtiles[nm] = dram.tile(list(shp), f32, kind="ExternalInput", name=nm,
                                   uniquify=False)
            names[nm] = nm
            sb[nm] = sing.tile(list(shp), f32, name="sb_" + nm)
            nc.sync.dma_start(sb[nm][:], dtiles[nm][:])

        out_t = dram.tile([M, 6], f32, kind="ExternalOutput", name="out",
                          uniquify=False)
        names["out"] = "out"

        # staging: positions and velocities, one column per step in the body
        stP = sing.tile([3, U], f32, name="stP")
        stV = sing.tile([3, U], f32, name="stV")
        nc.sync.dma_start(stP[0:3, U - 1:U], dtiles["s0t"][0:3, 0:1])
        nc.sync.dma_start(stV[0:3, U - 1:U], dtiles["s0t"][3:6, 0:1])

        # acceleration history for the stage-state predictor
        accH1 = sing.tile([3, 4], f32, name="accH1")  # step i-1 accs
        accH2 = sing.tile([3, 4], f32, name="accH2")  # step i-2 accs
        nc.vector.memset(accH1[:], 0.0)
        nc.vector.memset(accH2[:], 0.0)

        ones14 = sing.tile([1, 4], f32, name="ones14")
        nc.vector.memset(ones14[:], 1.0)

        # lane-paired gravity workspace (stage a: lanes 0-15, b: 16-31)
        Wk = sing.tile([1, 32], f32, name="Wk")
        nc.vector.memset(Wk[0:1, 0:1], 1.0)
        nc.vector.memset(Wk[0:1, 16:17], 1.0)
        WR = Wk[0:1, 0:32].rearrange("p (s l) -> p s l", s=2)

        V = nc.vector

        def mlp_batched(SPp, SPv, tag):
            psH1 = psum.tile([32, 4], f32, tag="mlp", name="psH1" + tag)
            nc.tensor.matmul(psH1[:], sb["w1spt"][:], SPp[:], start=True,
                             stop=False)
            nc.tensor.matmul(psH1[:], sb["w1svt"][:], SPv[:], start=False,
                             stop=True)
            h1 = work.tile([32, 4], f32, tag="h1", name="h1" + tag)
            nc.scalar.activation(h1[:], psH1[:], Act.Tanh, bias=sb["b1t"][:])
            psH2 = psum.tile([32, 4], f32, tag="mlp", name="psH2" + tag)
            nc.tensor.matmul(psH2[:], sb["w2t"][:], h1[:], start=True, stop=True)
            h2t = work.tile([32, 4], f32, tag="h2", name="h2" + tag)
            nc.scalar.activation(h2t[:], psH2[:], Act.Tanh, bias=sb["b2t"][:])
            psNN = psum.tile([3, 4], f32, tag="mlp", name="psNN" + tag)
            nc.tensor.matmul(psNN[:], sb["w3et"][:], h2t[:], start=True,
                             stop=False)
            nc.tensor.matmul(psNN[:], sb["b3t"][:], ones14[:], start=False,
                             stop=True)
            return psNN

        def pair_chain(psF, tag):
            def P(off, n=1, step=1):
                return WR[0:1, 0:2, ds(off, n, step)]

            psFR = psF[0:1, 0:32].rearrange("p (s l) -> p s l", s=2)
            V.tensor_copy(P(13, 3), psFR[0:1, 0:2, 0:3])
            r2 = P(13)
            V.scalar_tensor_tensor(P(10), r2, C1Q_RSQ, r2, Alu.add, Alu.mult)
            V.scalar_tensor_tensor(P(11), P(10), C2_RSQ,
                                   sb["c0t"][0:1, 0:1].to_broadcast((1, 2, 1)),
                                   Alu.mult, Alu.add)
            V.tensor_tensor(P(12), P(11), P(11), Alu.mult)
            V.scalar_tensor_tensor(P(8), P(12), -0.5, r2, Alu.mult, Alu.mult)
            V.scalar_tensor_tensor(P(9), P(8), 1.5, P(11), Alu.add, Alu.mult)
            V.tensor_tensor(P(12), P(9), P(9), Alu.mult)
            V.scalar_tensor_tensor(P(8), P(12), -0.5, r2, Alu.mult, Alu.mult)
            V.scalar_tensor_tensor(P(9), P(8), 1.5, P(9), Alu.add, Alu.mult)
            V.tensor_tensor(P(1), P(9), P(9), Alu.mult)
            u_b2 = P(1).to_broadcast((1, 2, 2))
            V.tensor_tensor(P(5, 2, 2), P(14, 2), u_b2, Alu.mult)
            V.tensor_tensor(P(2, 2), P(1, 2, 6), u_b2, Alu.mult)
            V.tensor_tensor(P(6), P(1), P(9), Alu.mult)
            V.tensor_tensor(P(4), P(2), P(7), Alu.mult)

        def pair_head(posA, posB, tag):
            """posA/posB: (3,1) SBUF APs. DVE squares + 4 mmF -> psF."""
            sq2 = work.tile([3, 2], f32, tag="sq2", name="sq2" + tag)
            V.tensor_tensor(sq2[0:3, 0:1], posA, posA, Alu.mult)
            V.tensor_tensor(sq2[0:3, 1:2], posB, posB, Alu.mult)
            psF = psum.tile([1, 32], f32, tag="gp", name="psF" + tag)
            nc.tensor.matmul(psF[0:1, 0:2], sq2[0:3, 0:1], sb["cmv"][0:3, 0:2],
                             start=True, stop=True)
            nc.tensor.matmul(psF[0:1, 2:3], posA, sb["cmv"][0:3, 2:3],
                             start=True, stop=True)
            nc.tensor.matmul(psF[0:1, 16:18], sq2[0:3, 1:2], sb["cmv"][0:3, 0:2],
                             start=True, stop=True)
            nc.tensor.matmul(psF[0:1, 18:19], posB, sb["cmv"][0:3, 2:3],
                             start=True, stop=True)
            return psF

        def gtail_pre(widx, pos_p, tag):
            """everything except the final acc combine; returns (Svec, psB)."""
            psB = psum.tile([3, 8], f32, tag="gp", name="psB" + tag)
            nc.tensor.matmul(psB[0:3, 0:8], sb["ones13"][0:1, 0:3],
                             Wk[0:1, widx:widx + 8], start=True, stop=True)
            pip = work.tile([3, 7], f32, tag="pip", name="pip" + tag)
            V.tensor_scalar(pip[0:3, 0:5], psB[0:3, 0:5], pos_p, None, Alu.mult)
            nc.scalar.copy(pip[0:3, 5:7], psB[0:3, 1:3])
            Dt = work.tile([3, 7], f32, tag="Dt", name="Dt" + tag)
            V.scalar_tensor_tensor(Dt[:], sb["g1c"][:], psB[0:3, 5:6],
                                   sb["g0c"][:], Alu.mult, Alu.add)
            scr = work.tile([3, 7], f32, tag="scr", name="scr" + tag)
            V.tensor_tensor(scr[:], pip[:], Dt[:], Alu.mult)
            Svec = work.tile([3, 1], f32, tag="Svec", name="Svec" + tag)
            V.tensor_reduce(Svec[:], scr[:], axis=mybir.AxisListType.X,
                            op=Alu.add)
            return Svec, psB

        def acc_combine(Svec, psB, psNN, k):
            # accH1[:, k] = Svec * r3 + psNN[:, k]
            V.scalar_tensor_tensor(accH1[0:3, k:k + 1], Svec[:],
                                   psB[0:3, 6:7], psNN[0:3, k:k + 1],
                                   Alu.mult, Alu.add)

        def step(j, iv):
            t = f"_{j}"
            jp = (j - 1) % U
            posC = stP[0:3, jp:jp + 1]
            velC = stV[0:3, jp:jp + 1]
            # ---- predictor (DVE) ----
            G = nc.vector
            ahT = work.tile([3, 3], f32, tag="ahT", name="ahT" + t)
            G.scalar_tensor_tensor(ahT[:], accH1[0:3, 0:3], 2.0,
                                   accH2[0:3, 0:3], Alu.mult, Alu.subtract)
            G.tensor_copy(accH2[:], accH1[:])
            SPp = work.tile([3, 4], f32, tag="SPp", name="SPp" + t)
            SPv = work.tile([3, 4], f32, tag="SPv", name="SPv" + t)
            p4b = work.tile([3, 1], f32, tag="p4b", name="p4b" + t)
            V.scalar_tensor_tensor(SPp[0:3, 1:2], velC, h2, posC,
                                   Alu.mult, Alu.add)          # p2ex
            V.scalar_tensor_tensor(p4b[:], velC, hf, posC, Alu.mult, Alu.add)
            G.tensor_copy(SPp[0:3, 0:1], posC)
            G.scalar_tensor_tensor(SPp[0:3, 2:3], ahT[0:3, 0:1], h24,
                                   SPp[0:3, 1:2], Alu.mult, Alu.add)
            G.scalar_tensor_tensor(SPp[0:3, 3:4], ahT[0:3, 1:2], h22,
                                   p4b[:], Alu.mult, Alu.add)
            G.tensor_copy(SPv[0:3, 0:1], velC)
            G.scalar_tensor_tensor(SPv[0:3, 1:2], ahT[0:3, 0:1], h2, velC,
                                   Alu.mult, Alu.add)
            G.scalar_tensor_tensor(SPv[0:3, 2:3], ahT[0:3, 1:2], h2, velC,
                                   Alu.mult, Alu.add)
            G.scalar_tensor_tensor(SPv[0:3, 3:4], ahT[0:3, 2:3], hf, velC,
                                   Alu.mult, Alu.add)
            # ---- batched MLP (PE/ACT, runs concurrently) ----
            psNN = mlp_batched(SPp, SPv, t)
            # ---- gravity pair (1,2): pos1=posC, pos2=SPp[:,1] (exact) ----
            psF12 = pair_head(posC, SPp[0:3, 1:2], "a" + t)
            pair_chain(psF12, "a" + t)
            Sv1, psB1 = gtail_pre(0, posC, "s1" + t)
            Sv2, psB2 = gtail_pre(16, SPp[0:3, 1:2], "s2" + t)
            acc_combine(Sv1, psB1, psNN, 0)
            acc_combine(Sv2, psB2, psNN, 1)
            # ---- exact positions for stages 3,4 ----
            pos3 = work.tile([3, 1], f32, tag="pos3", name="pos3" + t)
            V.scalar_tensor_tensor(pos3[:], accH1[0:3, 0:1], h24,
                                   SPp[0:3, 1:2], Alu.mult, Alu.add)
            pos4 = work.tile([3, 1], f32, tag="pos4", name="pos4" + t)
            V.scalar_tensor_tensor(pos4[:], accH1[0:3, 1:2], h22, p4b[:],
                                   Alu.mult, Alu.add)
            # ---- gravity pair (3,4) ----
            psF34 = pair_head(pos3[0:3, 0:1], pos4[0:3, 0:1], "b" + t)
            pair_chain(psF34, "b" + t)
            Sv3, psB3 = gtail_pre(0, pos3[0:3, 0:1], "s3" + t)
            Sv4, psB4 = gtail_pre(16, pos4[0:3, 0:1], "s4" + t)
            acc_combine(Sv3, psB3, psNN, 2)
            acc_combine(Sv4, psB4, psNN, 3)
            # ---- commit ----
            t12 = work.tile([3, 1], f32, tag="t12", name="t12" + t)
            V.tensor_tensor(t12[:], accH1[0:3, 0:1], accH1[0:3, 1:2], Alu.add)
            t123 = work.tile([3, 1], f32, tag="t123", name="t123" + t)
            V.tensor_tensor(t123[:], t12[:], accH1[0:3, 2:3], Alu.add)
            V.scalar_tensor_tensor(stP[0:3, j:j + 1], t123[:], h26, p4b[:],
                                   Alu.mult, Alu.add)
            s12 = work.tile([3, 1], f32, tag="s12", name="s12" + t)
            V.scalar_tensor_tensor(s12[:], accH1[0:3, 1:2], 2.0,
                                   accH1[0:3, 0:1], Alu.mult, Alu.add)
            s34 = work.tile([3, 1], f32, tag="s34", name="s34" + t)
            V.scalar_tensor_tensor(s34[:], accH1[0:3, 2:3], 2.0,
                                   accH1[0:3, 3:4], Alu.mult, Alu.add)
            ssum = work.tile([3, 1], f32, tag="ssum", name="ssum" + t)
            V.tensor_tensor(ssum[:], s12[:], s34[:], Alu.add)
            V.scalar_tensor_tensor(stV[0:3, j:j + 1], ssum[:], h6, velC,
                                   Alu.mult, Alu.add)

        with tc.For_i(0, M, U) as iv:
            for j in range(U):
                step(j, iv)
                if j == H - 1 or j == U - 1:
                    lo = 0 if j == H - 1 else H
                    nc.sync.dma_start(
                        out_t[ds(iv + lo, H), 0:3].rearrange("r c -> c r"),
                        stP[0:3, lo:lo + H])
                    nc.sync.dma_start(
                        out_t[ds(iv + lo, H), 3:6].rearrange("r c -> c r"),
                        stV[0:3, lo:lo + H])

    nc.compile()
    return nc, names


_CACHE = {}


def _get_built(M, U=8, dt=60.0):
    key = (M, U, float(dt))
    if key not in _CACHE:
        _CACHE[key] = build(M, U, dt)
    return _CACHE[key]


def kernel(state0, eval_times, W1, b1, W2, b2, W3, b3, log_scale, t0, dt,
           **extra):
    M = int(np.asarray(eval_times).shape[0])
    U = 8 if M % 8 == 0 else 2
    nc, names = _get_built(M, U, float(np.asarray(dt)))
    hd = make_host_inputs(np.asarray(state0), np.asarray(W1), np.asarray(b1),
                          np.asarray(W2), np.asarray(b2), np.asarray(W3),
                          np.asarray(b3), np.asarray(log_scale),
                          float(np.asarray(dt)))
    in_map = {names[k]: np.ascontiguousarray(v) for k, v in hd.items()}
    from concourse import bass_utils
    res = bass_utils.run_bass_kernel_spmd(
        nc, [dict(in_map) for _ in range(8)], core_ids=list(range(8)))
    out = res.results[0][names["out"]]
    return np.asarray(out, dtype=np.float32)


if __name__ == "__main__":
    # quick CoreSim functional check on a short horizon
    M = 8
    nc, names = build(M=M, U=M)
    import jax
    jax.config.update('jax_platforms', 'cpu')
    import reference as Rf
    inputs = Rf.setup_inputs()
    hd = make_host_inputs(np.asarray(inputs['state0']), np.asarray(inputs['W1']),
                          np.asarray(inputs['b1']), np.asarray(inputs['W2']),
                          np.asarray(inputs['b2']), np.asarray(inputs['W3']),
                          np.asarray(inputs['b3']),
                          np.asarray(inputs['log_scale']), float(inputs['dt']))
    from concourse.bass_interp import CoreSim
    sim = CoreSim(nc)
    for k, v in hd.items():
        sim.tensor(names[k])[:] = v
    sim.simulate()
    got = np.array(sim.tensor(names["out"]))
    exp = np.asarray(Rf.reference(**inputs))[:M]
    err = np.abs(got - exp)
    rel = np.linalg.norm(got - exp) / np.linalg.norm(exp)
    print("sim out[0]:", got[0])
    print("ref    [0]:", exp[0])
    print("sim out[-1]:", got[-1])
    print("ref     [-1]:", exp[-1])
    print(f"fro rel err over {M} steps: {rel:.3e}  absmax {err.max():.3e}")
    print("sim predicted time (ns):", sim.time)


# revision 17
# speedup vs baseline: 1.0565x; 1.0565x over previous
"""Trainium2 Bass kernel for nn_NeuralODE (RK4 satellite propagation, M=10000 steps).

Self-contained: takes FULL inputs (as produced by the problem's setup_inputs),
runs the sequential RK4 integration on NeuronCore 0 (replicated across cores
0-7 via SPMD), returns the full (M, 6) trajectory.

Design notes:
  - state (6,1) on partitions 0-5. MLP (6->32->32->3 tanh) on TensorE+ScalarE.
  - gravity J2..J5 evaluated via a factored form:
      g = r3inv * ( pos * S + e_z * R )
      S = -MU + u*D2(m) + u^2*D4(m) + (z*u^2)*D3(m) + (z*u^3)*D5(m)
      (m = z^2*u, u = 1/r^2; D_k linear in m -- the dropped m^2 terms
       contribute < 1e-9 relative)
    computed as a 5-lane tensor_tensor_reduce with per-partition (x/y/z)
    coefficient tables; R folded into two extra lanes of the z row.
  - 1/r via quadratic polynomial init (fitted on r^2 in [3.9e7, 5.1e7]) plus
    two Newton iterations (validated to sit within the f32 reference's own
    rounding envelope).
  - RK4 stage states & next-state built as tiny PE matmul accumulations:
      s2 = [p0 + (h/2)v0 ; v0 + (h/2)a1]
      s3 = [p2 + (h^2/4)a1 ; v0 + (h/2)a2]
      s4 = [p0 + h v0 + (h^2/2)a2 ; v0 + h a3]
      s' = [p0 + h v0 + (h^2/6)(a1+a2+a3) ; v0 + (h/6)(a1+2a2+2a3+a4)]
  - all dt-dependent constants are folded on the host into input tensors, so
    one compiled NEFF serves any dt.
"""
import os
import numpy as np
from contextlib import ExitStack

MU = 398600.4418
RE = 6378.137
J2 = 1.08262668e-3
J3 = -2.53265648e-6
J4 = -1.61962159e-6
J5 = -2.27296082e-7
R_REF = 7000.0
V_REF = 7.5
A_REF = V_REF * V_REF / R_REF

F32 = np.float32


# ----------------------------------------------------------------------------
# host-side constant folding
# ----------------------------------------------------------------------------

def _fit_rsqrt_quad(lo=3.9e7, hi=5.1e7):
    """least-squares quadratic fit of x^{-1/2} on [lo, hi] (Chebyshev nodes)."""
    xs = (np.cos(np.pi * (np.arange(4000) + 0.5) / 4000) + 1) / 2 * (hi - lo) + lo
    f = xs ** -0.5
    t = 2 * (xs - lo) / (hi - lo) - 1
    V = np.polynomial.chebyshev.chebvander(t, 2)
    c, *_ = np.linalg.lstsq(V, f, rcond=None)
    pt = np.polynomial.chebyshev.cheb2poly(c)
    a = 2 / (hi - lo)
    b = -1 - 2 * lo / (hi - lo)
    px = np.zeros(3)
    for k, ck in enumerate(pt):
        poly = np.array([1.0])
        for _ in range(k):
            poly = np.convolve(poly, np.array([b, a]))
        px[:len(poly)] += ck * poly
    return px  # [c0, c1, c2]: c0 + c1*x + c2*x^2


_PX = _fit_rsqrt_quad()
C0_RSQ = float(_PX[0])
C1_RSQ = float(_PX[1])
C2_RSQ = float(_PX[2])
C1Q_RSQ = C1_RSQ / C2_RSQ  # v0 = C2*((r2 + C1Q)*r2) + C0


def _gravity_tables():
    """(G1, G0) each (3, 7): D = G1*m + G0 per lane; lanes
    [const(-MU), u*D2, u2*D4, (u*zu)*D3, (u2*zu)*D5, R1(u), R2(u2)]."""
    G0 = np.zeros((3, 7))
    G1 = np.zeros((3, 7))
    G0[:, 0] = -MU
    c2 = J2 * MU * RE**2
    G0[:2, 1] = -1.5 * c2
    G1[:2, 1] = 7.5 * c2
    G0[2, 1] = -4.5 * c2
    G1[2, 1] = 7.5 * c2
    c4 = (15.0 / 8.0) * J4 * MU * RE**4
    G0[:2, 2] = c4
    G1[:2, 2] = -14.0 * c4
    G0[2, 2] = 5.0 * c4
    G1[2, 2] = -(70.0 / 3.0) * c4
    c3 = -2.5 * J3 * MU * RE**3
    G0[:2, 3] = 3.0 * c3
    G1[:2, 3] = -7.0 * c3
    G0[2, 3] = 6.0 * c3
    G1[2, 3] = -7.0 * c3
    c5 = (3.0 / 8.0) * J5 * MU * RE**5
    G0[:2, 4] = 35.0 * c5
    G1[:2, 4] = -210.0 * c5
    G0[2, 4] = 105.0 * c5
    G1[2, 4] = -315.0 * c5
    # R lanes (z row only): R = rho1*u + rho2*u2
    G0[2, 5] = 1.5 * J3 * MU * RE**3
    G0[2, 6] = -1.875 * J5 * MU * RE**5
    return F32(G1), F32(G0)


def _amat(alpha):
    """lhsT for out = [[I, alpha*I],[0, I]] @ s, split into pos/vel halves."""
    A = np.zeros((6, 6))
    A[:3, :3] = np.eye(3)
    A[:3, 3:] = alpha * np.eye(3)
    A[3:, 3:] = np.eye(3)
    Ap = A.copy(); Ap[3:, :] = 0.0           # pos rows only
    Av = A.copy(); Av[:3, :] = 0.0           # vel rows only
    return F32(Ap.T), F32(Av.T), F32(A.T)


def _bmat(alpha, beta):
    """lhsT (3,6) for out(6,1) = [alpha*a ; beta*a], a (3,1)."""
    B = np.zeros((6, 3))
    B[:3, :] = alpha * np.eye(3)
    B[3:, :] = beta * np.eye(3)
    return F32(B.T)


def make_host_inputs(state0, W1, b1, W2, b2, W3, b3, log_scale, dt):
    h = float(dt)
    scale_vec = np.array([1 / R_REF] * 3 + [1 / V_REF] * 3, np.float64)
    W1s = F32(np.float64(W1) * scale_vec[None, :])
    amp = F32(F32(np.exp(F32(log_scale))) * F32(A_REF))
    W3e = F32(np.float64(W3) * np.float64(amp))
    b3e = F32(np.float64(b3) * np.float64(amp))
    G1, G0 = _gravity_tables()
    d = {
        "w1spt": np.ascontiguousarray(W1s[:, 0:3].T),   # (3,32)
        "w1svt": np.ascontiguousarray(W1s[:, 3:6].T),   # (3,32)
        "w2t": np.ascontiguousarray(F32(W2).T),         # (32,32)
        "w3et": np.ascontiguousarray(W3e.T),            # (32,3)
        "b1t": F32(b1).reshape(32, 1),
        "b2t": F32(b2).reshape(32, 1),
        "b3t": b3e.reshape(1, 3),
        "g1c": G1, "g0c": G0,                           # (3,7)
        "c0t": F32([[C0_RSQ]] * 3),                     # (3,1)
        "cmv": F32([[1, 0, 0], [1, 0, 0], [1, 1, 1]]),  # (3,3)
        "ones13": F32([[1, 1, 1]]),                     # (1,3)
        "s0t": F32(state0).reshape(6, 1),
    }
    return d


# ----------------------------------------------------------------------------
# kernel builder
# ----------------------------------------------------------------------------

def build(M=10000, U=8, dt=60.0):
    assert U % 2 == 0 and M % U == 0
    return _build_inner(M, U, U // 2, float(dt))


def _build_inner(M, U, H, h):
    import concourse.bacc as bacc
    import concourse.tile as tile
    from concourse import mybir
    from concourse.bass import ds

    f32 = mybir.dt.float32
    Alu = mybir.AluOpType
    Act = mybir.ActivationFunctionType

    hf = float(F32(h))
    h2 = float(F32(hf / 2))
    h6 = float(F32(hf / 6))
    hh = float(F32(F32(hf) * F32(hf)))
    h24 = float(F32(hh / 4))
    h22 = float(F32(hh / 2))
    h26 = float(F32(hh / 6))

    nc = bacc.Bacc(None, target_bir_lowering=False, debug=False)
    names = {}

    with tile.TileContext(nc) as tc, ExitStack() as ctx:
        dram = ctx.enter_context(tc.tile_pool(name="dram", bufs=1, space="DRAM"))
        sing = ctx.enter_context(tc.tile_pool(name="sing", bufs=1))
        work = ctx.enter_context(tc.tile_pool(name="work", bufs=2))
        psum = ctx.enter_context(tc.tile_pool(name="ps", bufs=2, space="PSUM"))

        shapes = {
            "w1spt": (3, 32), "w1svt": (3, 32), "w2t": (32, 32),
            "w3et": (32, 3), "b1t": (32, 1), "b2t": (32, 1), "b3t": (1, 3),
            "g1c": (3, 7), "g0c": (3, 7), "c0t": (3, 1), "cmv": (3, 3),
            "ones13": (1, 3), "s0t": (6, 1),
        }
        dtiles = {}
        sb = {}
        for nm, shp in shapes.items():
            dtiles[nm] = dram.tile(list(shp), f32, kind="ExternalInput", name=nm,
                                   uniquify=False)
            names[nm] = nm
            sb[nm] = sing.tile(list(shp), f32, name="sb_" + nm)
            nc.sync.dma_start(sb[nm][:], dtiles[nm][:])

        out_t = dram.tile([M, 6], f32, kind="ExternalOutput", name="out",
                          uniquify=False)
        names["out"] = "out"

        # staging: positions and velocities, one column per step in the body
        stP = sing.tile([3, U], f32, name="stP")
        stV = sing.tile([3, U], f32, name="stV")
        nc.sync.dma_start(stP[0:3, U - 1:U], dtiles["s0t"][0:3, 0:1])
        nc.sync.dma_start(stV[0:3, U - 1:U], dtiles["s0t"][3:6, 0:1])

        # acceleration history for the stage-state predictor
        accH1 = sing.tile([3, 4], f32, name="accH1")  # step i-1 accs
        accH2 = sing.tile([3, 4], f32, name="accH2")  # step i-2 accs
        nc.vector.memset(accH1[:], 0.0)
        nc.vector.memset(accH2[:], 0.0)

        ones14 = sing.tile([1, 4], f32, name="ones14")
        nc.vector.memset(ones14[:], 1.0)

        # lane-paired gravity workspaces (3-partition replicated rows;
        # stage a: lanes 0-15, b: 16-31); one per pair to decouple WARs
        Wk12 = sing.tile([3, 32], f32, name="Wk12")
        Wk34 = sing.tile([3, 32], f32, name="Wk34")
        for _w in (Wk12, Wk34):
            nc.vector.memset(_w[0:3, 0:1], 1.0)
            nc.vector.memset(_w[0:3, 16:17], 1.0)

        V = nc.vector

        def mlp_batched(SPp, SPv, tag):
            psH1 = psum.tile([32, 4], f32, tag="mlp", name="psH1" + tag)
            nc.tensor.matmul(psH1[:], sb["w1spt"][:], SPp[:], start=True,
                             stop=False)
            nc.tensor.matmul(psH1[:], sb["w1svt"][:], SPv[:], start=False,
                             stop=True)
            h1 = work.tile([32, 4], f32, tag="h1", name="h1" + tag)
            nc.scalar.activation(h1[:], psH1[:], Act.Tanh, bias=sb["b1t"][:])
            psH2 = psum.tile([32, 4], f32, tag="mlp", name="psH2" + tag)
            nc.tensor.matmul(psH2[:], sb["w2t"][:], h1[:], start=True, stop=True)
            h2t = work.tile([32, 4], f32, tag="h2", name="h2" + tag)
            nc.scalar.activation(h2t[:], psH2[:], Act.Tanh, bias=sb["b2t"][:])
            psNN = psum.tile([3, 4], f32, tag="mlp", name="psNN" + tag)
            nc.tensor.matmul(psNN[:], sb["w3et"][:], h2t[:], start=True,
                             stop=False)
            nc.tensor.matmul(psNN[:], sb["b3t"][:], ones14[:], start=False,
                             stop=True)
            return psNN

        def pair_chain(Wk, psF, tag):
            WR = Wk[0:3, 0:32].rearrange("p (s l) -> p s l", s=2)

            def P(off, n=1, step=1):
                return WR[0:3, 0:2, ds(off, n, step)]

            psFR = psF[0:3, 0:32].rearrange("p (s l) -> p s l", s=2)
            V.tensor_copy(P(13, 3), psFR[0:3, 0:2, 0:3])
            r2 = P(13)
            V.scalar_tensor_tensor(P(10), r2, C1Q_RSQ, r2, Alu.add, Alu.mult)
            V.scalar_tensor_tensor(P(11), P(10), C2_RSQ,
                                   sb["c0t"][0:3, 0:1].to_broadcast((3, 2, 1)),
                                   Alu.mult, Alu.add)
            V.tensor_tensor(P(12), P(11), P(11), Alu.mult)
            V.scalar_tensor_tensor(P(8), P(12), -0.5, r2, Alu.mult, Alu.mult)
            V.scalar_tensor_tensor(P(9), P(8), 1.5, P(11), Alu.add, Alu.mult)
            V.tensor_tensor(P(12), P(9), P(9), Alu.mult)
            V.scalar_tensor_tensor(P(8), P(12), -0.5, r2, Alu.mult, Alu.mult)
            V.scalar_tensor_tensor(P(9), P(8), 1.5, P(9), Alu.add, Alu.mult)
            V.tensor_tensor(P(1), P(9), P(9), Alu.mult)
            u_b2 = P(1).to_broadcast((3, 2, 2))
            V.tensor_tensor(P(5, 2, 2), P(14, 2), u_b2, Alu.mult)
            V.tensor_tensor(P(2, 2), P(1, 2, 6), u_b2, Alu.mult)
            V.tensor_tensor(P(6), P(1), P(9), Alu.mult)
            V.tensor_tensor(P(4), P(2), P(7), Alu.mult)

        def pair_head(posA, posB, tag):
            """posA/posB: (3,1) SBUF APs. DVE squares + 4 mmF -> psF with
            rows replicated across 3 partitions (free-broadcast stationary)."""
            sq2 = work.tile([3, 2], f32, tag="sq2", name="sq2" + tag)
            V.tensor_tensor(sq2[0:3, 0:1], posA, posA, Alu.mult)
            V.tensor_tensor(sq2[0:3, 1:2], posB, posB, Alu.mult)
            psF = psum.tile([3, 32], f32, tag="gp", name="psF" + tag)
            nc.tensor.matmul(psF[0:3, 0:2], sq2[0:3, 0:1].to_broadcast((3, 3)),
                             sb["cmv"][0:3, 0:2], start=True, stop=True)
            nc.tensor.matmul(psF[0:3, 2:3], posA.to_broadcast((3, 3)),
                             sb["cmv"][0:3, 2:3], start=True, stop=True)
            nc.tensor.matmul(psF[0:3, 16:18], sq2[0:3, 1:2].to_broadcast((3, 3)),
                             sb["cmv"][0:3, 0:2], start=True, stop=True)
            nc.tensor.matmul(psF[0:3, 18:19], posB.to_broadcast((3, 3)),
                             sb["cmv"][0:3, 2:3], start=True, stop=True)
            return psF

        def gtail_pair(Wk, posa_p, posb_p, psNN, ka, kb, tag):
            """merged tails for one pair; reads chain lanes from Wk (SBUF),
            writes accs into accH1 cols ka, kb."""
            pip = work.tile([3, 14], f32, tag="pip", name="pip" + tag)
            pipR = pip[0:3, 0:14].rearrange("p (s l) -> p s l", s=2)
            V.tensor_scalar(pip[0:3, 0:5], Wk[0:3, 0:5], posa_p, None, Alu.mult)
            V.tensor_scalar(pip[0:3, 7:12], Wk[0:3, 16:21], posb_p, None,
                            Alu.mult)
            WkR = Wk[0:3, 0:32].rearrange("p (s l) -> p s l", s=2)
            nc.scalar.copy(pipR[0:3, 0:2, 5:7], WkR[0:3, 0:2, 1:3])
            Dt = work.tile([3, 14], f32, tag="Dt", name="Dt" + tag)
            V.scalar_tensor_tensor(Dt[0:3, 0:7], sb["g1c"][:], Wk[0:3, 5:6],
                                   sb["g0c"][:], Alu.mult, Alu.add)
            V.scalar_tensor_tensor(Dt[0:3, 7:14], sb["g1c"][:], Wk[0:3, 21:22],
                                   sb["g0c"][:], Alu.mult, Alu.add)
            scr = work.tile([3, 14], f32, tag="scr", name="scr" + tag)
            V.tensor_tensor(scr[:], pip[:], Dt[:], Alu.mult)
            Svec = work.tile([3, 2], f32, tag="Svec", name="Svec" + tag)
            V.tensor_reduce(Svec[:], scr[0:3, 0:14].rearrange(
                "p (s l) -> p s l", s=2), axis=mybir.AxisListType.X, op=Alu.add)
            V.scalar_tensor_tensor(accH1[0:3, ka:ka + 1], Svec[0:3, 0:1],
                                   Wk[0:3, 6:7], psNN[0:3, ka:ka + 1],
                                   Alu.mult, Alu.add)
            V.scalar_tensor_tensor(accH1[0:3, kb:kb + 1], Svec[0:3, 1:2],
                                   Wk[0:3, 22:23], psNN[0:3, kb:kb + 1],
                                   Alu.mult, Alu.add)

        def step(j, iv):
            t = f"_{j}"
            jp = (j - 1) % U
            posC = stP[0:3, jp:jp + 1]
            velC = stV[0:3, jp:jp + 1]
            # ---- predictor (DVE) ----
            G = nc.vector
            ahT = work.tile([3, 3], f32, tag="ahT", name="ahT" + t)
            G.scalar_tensor_tensor(ahT[:], accH1[0:3, 0:3], 2.0,
                                   accH2[0:3, 0:3], Alu.mult, Alu.subtract)
            G.tensor_copy(accH2[:], accH1[:])
            SPp = work.tile([3, 4], f32, tag="SPp", name="SPp" + t)
            SPv = work.tile([3, 4], f32, tag="SPv", name="SPv" + t)
            p4b = work.tile([3, 1], f32, tag="p4b", name="p4b" + t)
            V.scalar_tensor_tensor(SPp[0:3, 1:2], velC, h2, posC,
                                   Alu.mult, Alu.add)          # p2ex
            V.scalar_tensor_tensor(p4b[:], velC, hf, posC, Alu.mult, Alu.add)
            G.tensor_copy(SPp[0:3, 0:1], posC)
            G.scalar_tensor_tensor(SPp[0:3, 2:3], ahT[0:3, 0:1], h24,
                                   SPp[0:3, 1:2], Alu.mult, Alu.add)
            G.scalar_tensor_tensor(SPp[0:3, 3:4], ahT[0:3, 1:2], h22,
                                   p4b[:], Alu.mult, Alu.add)
            G.tensor_copy(SPv[0:3, 0:1], velC)
            G.scalar_tensor_tensor(SPv[0:3, 1:2], ahT[0:3, 0:1], h2, velC,
                                   Alu.mult, Alu.add)
            G.scalar_tensor_tensor(SPv[0:3, 2:3], ahT[0:3, 1:2], h2, velC,
                                   Alu.mult, Alu.add)
            G.scalar_tensor_tensor(SPv[0:3, 3:4], ahT[0:3, 2:3], hf, velC,
                                   Alu.mult, Alu.add)
            # ---- batched MLP (PE/ACT, runs concurrently) ----
            psNN = mlp_batched(SPp, SPv, t)
            # ---- gravity pair (1,2): pos1=posC, pos2=SPp[:,1] (exact) ----
            psF12 = pair_head(posC, SPp[0:3, 1:2], "a" + t)
            pair_chain(Wk12, psF12, "a" + t)
            gtail_pair(Wk12, posC, SPp[0:3, 1:2], psNN, 0, 1, "x" + t)
            # ---- exact positions for stages 3,4 ----
            pos3 = work.tile([3, 1], f32, tag="pos3", name="pos3" + t)
            V.scalar_tensor_tensor(pos3[:], accH1[0:3, 0:1], h24,
                                   SPp[0:3, 1:2], Alu.mult, Alu.add)
            pos4 = work.tile([3, 1], f32, tag="pos4", name="pos4" + t)
            V.scalar_tensor_tensor(pos4[:], accH1[0:3, 1:2], h22, p4b[:],
                                   Alu.mult, Alu.add)
            # ---- gravity pair (3,4) ----
            psF34 = pair_head(pos3[0:3, 0:1], pos4[0:3, 0:1], "b" + t)
            pair_chain(Wk34, psF34, "b" + t)
            gtail_pair(Wk34, pos3[0:3, 0:1], pos4[0:3, 0:1], psNN, 2, 3,
                       "y" + t)
            # ---- commit (structured so only ONE op follows a4) ----
            t12 = work.tile([3, 1], f32, tag="t12", name="t12" + t)
            V.tensor_tensor(t12[:], accH1[0:3, 0:1], accH1[0:3, 1:2], Alu.add)
            s12 = work.tile([3, 1], f32, tag="s12", name="s12" + t)
            V.scalar_tensor_tensor(s12[:], accH1[0:3, 1:2], 2.0,
                                   accH1[0:3, 0:1], Alu.mult, Alu.add)
            t123 = work.tile([3, 1], f32, tag="t123", name="t123" + t)
            V.tensor_tensor(t123[:], t12[:], accH1[0:3, 2:3], Alu.add)
            V.scalar_tensor_tensor(stP[0:3, j:j + 1], t123[:], h26, p4b[:],
                                   Alu.mult, Alu.add)
            s123 = work.tile([3, 1], f32, tag="s123", name="s123" + t)
            V.scalar_tensor_tensor(s123[:], accH1[0:3, 2:3], 2.0, s12[:],
                                   Alu.mult, Alu.add)
            vpre = work.tile([3, 1], f32, tag="vpre", name="vpre" + t)
            V.scalar_tensor_tensor(vpre[:], s123[:], h6, velC,
                                   Alu.mult, Alu.add)
            V.scalar_tensor_tensor(stV[0:3, j:j + 1], accH1[0:3, 3:4], h6,
                                   vpre[:], Alu.mult, Alu.add)

        with tc.For_i(0, M, U) as iv:
            for j in range(U):
                step(j, iv)
                if j == H - 1 or j == U - 1:
                    lo = 0 if j == H - 1 else H
                    nc.sync.dma_start(
                        out_t[ds(iv + lo, H), 0:3].rearrange("r c -> c r"),
                        stP[0:3, lo:lo + H])
                    nc.sync.dma_start(
                        out_t[ds(iv + lo, H), 3:6].rearrange("r c -> c r"),
                        stV[0:3, lo:lo + H])

    nc.compile()
    return nc, names


_CACHE = {}


def _get_built(M, U=8, dt=60.0):
    key = (M, U, float(dt))
    if key not in _CACHE:
        _CACHE[key] = build(M, U, dt)
    return _CACHE[key]


def kernel(state0, eval_times, W1, b1, W2, b2, W3, b3, log_scale, t0, dt,
           **extra):
    M = int(np.asarray(eval_times).shape[0])
    U = 16 if M % 16 == 0 else (8 if M % 8 == 0 else 2)
    nc, names = _get_built(M, U, float(np.asarray(dt)))
    hd = make_host_inputs(np.asarray(state0), np.asarray(W1), np.asarray(b1),
                          np.asarray(W2), np.asarray(b2), np.asarray(W3),
                          np.asarray(b3), np.asarray(log_scale),
                          float(np.asarray(dt)))
    in_map = {names[k]: np.ascontiguousarray(v) for k, v in hd.items()}
    from concourse import bass_utils
    res = bass_utils.run_bass_kernel_spmd(
        nc, [dict(in_map) for _ in range(8)], core_ids=list(range(8)))
    out = res.results[0][names["out"]]
    return np.asarray(out, dtype=np.float32)


if __name__ == "__main__":
    # quick CoreSim functional check on a short horizon
    M = 8
    nc, names = build(M=M, U=M)
    import jax
    jax.config.update('jax_platforms', 'cpu')
    import reference as Rf
    inputs = Rf.setup_inputs()
    hd = make_host_inputs(np.asarray(inputs['state0']), np.asarray(inputs['W1']),
                          np.asarray(inputs['b1']), np.asarray(inputs['W2']),
                          np.asarray(inputs['b2']), np.asarray(inputs['W3']),
                          np.asarray(inputs['b3']),
                          np.asarray(inputs['log_scale']), float(inputs['dt']))
    from concourse.bass_interp import CoreSim
    sim = CoreSim(nc)
    for k, v in hd.items():
        sim.tensor(names[k])[:] = v
    sim.simulate()
    got = np.array(sim.tensor(names["out"]))
    exp = np.asarray(Rf.reference(**inputs))[:M]
    err = np.abs(got - exp)
    rel = np.linalg.norm(got - exp) / np.linalg.norm(exp)
    print("sim out[0]:", got[0])
    print("ref    [0]:", exp[0])
    print("sim out[-1]:", got[-1])
    print("ref     [-1]:", exp[-1])
    print(f"fro rel err over {M} steps: {rel:.3e}  absmax {err.max():.3e}")
    print("sim predicted time (ns):", sim.time)


# revision 18
# speedup vs baseline: 1.3949x; 1.3203x over previous
"""Trainium2 Bass kernel for nn_NeuralODE (RK4 satellite propagation, M=10000 steps).

Self-contained: takes FULL inputs (as produced by the problem's setup_inputs),
runs the sequential RK4 integration on NeuronCore 0 (replicated across cores
0-7 via SPMD), returns the full (M, 6) trajectory.

Design notes:
  - state (6,1) on partitions 0-5. MLP (6->32->32->3 tanh) on TensorE+ScalarE.
  - gravity J2..J5 evaluated via a factored form:
      g = r3inv * ( pos * S + e_z * R )
      S = -MU + u*D2(m) + u^2*D4(m) + (z*u^2)*D3(m) + (z*u^3)*D5(m)
      (m = z^2*u, u = 1/r^2; D_k linear in m -- the dropped m^2 terms
       contribute < 1e-9 relative)
    computed as a 5-lane tensor_tensor_reduce with per-partition (x/y/z)
    coefficient tables; R folded into two extra lanes of the z row.
  - 1/r via quadratic polynomial init (fitted on r^2 in [3.9e7, 5.1e7]) plus
    two Newton iterations (validated to sit within the f32 reference's own
    rounding envelope).
  - RK4 stage states & next-state built as tiny PE matmul accumulations:
      s2 = [p0 + (h/2)v0 ; v0 + (h/2)a1]
      s3 = [p2 + (h^2/4)a1 ; v0 + (h/2)a2]
      s4 = [p0 + h v0 + (h^2/2)a2 ; v0 + h a3]
      s' = [p0 + h v0 + (h^2/6)(a1+a2+a3) ; v0 + (h/6)(a1+2a2+2a3+a4)]
  - all dt-dependent constants are folded on the host into input tensors, so
    one compiled NEFF serves any dt.
"""
import os
import numpy as np
from contextlib import ExitStack

MU = 398600.4418
RE = 6378.137
J2 = 1.08262668e-3
J3 = -2.53265648e-6
J4 = -1.61962159e-6
J5 = -2.27296082e-7
R_REF = 7000.0
V_REF = 7.5
A_REF = V_REF * V_REF / R_REF

F32 = np.float32


# ----------------------------------------------------------------------------
# host-side constant folding
# ----------------------------------------------------------------------------

def _fit_rsqrt_quad(lo=3.9e7, hi=5.1e7):
    """least-squares quadratic fit of x^{-1/2} on [lo, hi] (Chebyshev nodes)."""
    xs = (np.cos(np.pi * (np.arange(4000) + 0.5) / 4000) + 1) / 2 * (hi - lo) + lo
    f = xs ** -0.5
    t = 2 * (xs - lo) / (hi - lo) - 1
    V = np.polynomial.chebyshev.chebvander(t, 2)
    c, *_ = np.linalg.lstsq(V, f, rcond=None)
    pt = np.polynomial.chebyshev.cheb2poly(c)
    a = 2 / (hi - lo)
    b = -1 - 2 * lo / (hi - lo)
    px = np.zeros(3)
    for k, ck in enumerate(pt):
        poly = np.array([1.0])
        for _ in range(k):
            poly = np.convolve(poly, np.array([b, a]))
        px[:len(poly)] += ck * poly
    return px  # [c0, c1, c2]: c0 + c1*x + c2*x^2


_PX = _fit_rsqrt_quad()
C0_RSQ = float(_PX[0])
C1_RSQ = float(_PX[1])
C2_RSQ = float(_PX[2])
C1Q_RSQ = C1_RSQ / C2_RSQ  # v0 = C2*((r2 + C1Q)*r2) + C0


def _gravity_tables():
    """(G1, G0) each (3, 7): D = G1*m + G0 per lane; lanes
    [const(-MU), u*D2, u2*D4, (u*zu)*D3, (u2*zu)*D5, R1(u), R2(u2)]."""
    G0 = np.zeros((3, 7))
    G1 = np.zeros((3, 7))
    G0[:, 0] = -MU
    c2 = J2 * MU * RE**2
    G0[:2, 1] = -1.5 * c2
    G1[:2, 1] = 7.5 * c2
    G0[2, 1] = -4.5 * c2
    G1[2, 1] = 7.5 * c2
    c4 = (15.0 / 8.0) * J4 * MU * RE**4
    G0[:2, 2] = c4
    G1[:2, 2] = -14.0 * c4
    G0[2, 2] = 5.0 * c4
    G1[2, 2] = -(70.0 / 3.0) * c4
    c3 = -2.5 * J3 * MU * RE**3
    G0[:2, 3] = 3.0 * c3
    G1[:2, 3] = -7.0 * c3
    G0[2, 3] = 6.0 * c3
    G1[2, 3] = -7.0 * c3
    c5 = (3.0 / 8.0) * J5 * MU * RE**5
    G0[:2, 4] = 35.0 * c5
    G1[:2, 4] = -210.0 * c5
    G0[2, 4] = 105.0 * c5
    G1[2, 4] = -315.0 * c5
    # R lanes (z row only): R = rho1*u + rho2*u2
    G0[2, 5] = 1.5 * J3 * MU * RE**3
    G0[2, 6] = -1.875 * J5 * MU * RE**5
    return F32(G1), F32(G0)


def _amat(alpha):
    """lhsT for out = [[I, alpha*I],[0, I]] @ s, split into pos/vel halves."""
    A = np.zeros((6, 6))
    A[:3, :3] = np.eye(3)
    A[:3, 3:] = alpha * np.eye(3)
    A[3:, 3:] = np.eye(3)
    Ap = A.copy(); Ap[3:, :] = 0.0           # pos rows only
    Av = A.copy(); Av[:3, :] = 0.0           # vel rows only
    return F32(Ap.T), F32(Av.T), F32(A.T)


def _bmat(alpha, beta):
    """lhsT (3,6) for out(6,1) = [alpha*a ; beta*a], a (3,1)."""
    B = np.zeros((6, 3))
    B[:3, :] = alpha * np.eye(3)
    B[3:, :] = beta * np.eye(3)
    return F32(B.T)


def make_host_inputs(state0, W1, b1, W2, b2, W3, b3, log_scale, dt):
    h = float(dt)
    scale_vec = np.array([1 / R_REF] * 3 + [1 / V_REF] * 3, np.float64)
    W1s = F32(np.float64(W1) * scale_vec[None, :])
    amp = F32(F32(np.exp(F32(log_scale))) * F32(A_REF))
    W3e = F32(np.float64(W3) * np.float64(amp))
    b3e = F32(np.float64(b3) * np.float64(amp))
    G1, G0 = _gravity_tables()
    d = {
        "w1spt": np.ascontiguousarray(W1s[:, 0:3].T),   # (3,32)
        "w1svt": np.ascontiguousarray(W1s[:, 3:6].T),   # (3,32)
        "w2t": np.ascontiguousarray(F32(W2).T),         # (32,32)
        "w3et": np.ascontiguousarray(W3e.T),            # (32,3)
        "b1t": F32(b1).reshape(32, 1),
        "b2t": F32(b2).reshape(32, 1),
        "b3t": b3e.reshape(1, 3),
        "g1c": G1, "g0c": G0,                           # (3,7)
        "c0t": F32([[C0_RSQ]] * 3),                     # (3,1)
        "cmv": F32([[1, 0, 0], [1, 0, 0], [1, 1, 1]]),  # (3,3)
        "ones13": F32([[1, 1, 1]]),                     # (1,3)
        "s0t": F32(state0).reshape(6, 1),
    }
    return d


# ----------------------------------------------------------------------------
# kernel builder
# ----------------------------------------------------------------------------

def build(M=10000, U=8, dt=60.0):
    assert U % 2 == 0 and M % U == 0
    return _build_inner(M, U, U // 2, float(dt))


def _build_inner(M, U, H, h):
    import concourse.bacc as bacc
    import concourse.tile as tile
    from concourse import mybir
    from concourse.bass import ds

    f32 = mybir.dt.float32
    Alu = mybir.AluOpType
    Act = mybir.ActivationFunctionType

    hf = float(F32(h))
    h2 = float(F32(hf / 2))
    h6 = float(F32(hf / 6))
    hh = float(F32(F32(hf) * F32(hf)))
    h24 = float(F32(hh / 4))
    h22 = float(F32(hh / 2))
    h26 = float(F32(hh / 6))
    C3A = float(F32(3.0 * MU * h24))
    C3B = float(F32(-MU * h24))
    C4A = float(F32(3.0 * MU * h22))
    C4B = float(F32(-MU * h22))

    nc = bacc.Bacc(None, target_bir_lowering=False, debug=False)
    names = {}

    with tile.TileContext(nc) as tc, ExitStack() as ctx:
        dram = ctx.enter_context(tc.tile_pool(name="dram", bufs=1, space="DRAM"))
        sing = ctx.enter_context(tc.tile_pool(name="sing", bufs=1))
        work = ctx.enter_context(tc.tile_pool(name="work", bufs=2))
        psum = ctx.enter_context(tc.tile_pool(name="ps", bufs=2, space="PSUM"))

        shapes = {
            "w1spt": (3, 32), "w1svt": (3, 32), "w2t": (32, 32),
            "w3et": (32, 3), "b1t": (32, 1), "b2t": (32, 1), "b3t": (1, 3),
            "g1c": (3, 7), "g0c": (3, 7), "c0t": (3, 1), "cmv": (3, 3),
            "ones13": (1, 3), "s0t": (6, 1),
        }
        dtiles = {}
        sb = {}
        for nm, shp in shapes.items():
            dtiles[nm] = dram.tile(list(shp), f32, kind="ExternalInput", name=nm,
                                   uniquify=False)
            names[nm] = nm
            sb[nm] = sing.tile(list(shp), f32, name="sb_" + nm)
            nc.sync.dma_start(sb[nm][:], dtiles[nm][:])

        out_t = dram.tile([M, 6], f32, kind="ExternalOutput", name="out",
                          uniquify=False)
        names["out"] = "out"

        # staging: positions and velocities, one column per step in the body
        stP = sing.tile([3, U], f32, name="stP")
        stV = sing.tile([3, U], f32, name="stV")
        nc.sync.dma_start(stP[0:3, U - 1:U], dtiles["s0t"][0:3, 0:1])
        nc.sync.dma_start(stV[0:3, U - 1:U], dtiles["s0t"][3:6, 0:1])

        # acceleration history for the stage-state predictor
        accH1 = sing.tile([3, 4], f32, name="accH1")  # step i-1 accs
        accH2 = sing.tile([3, 4], f32, name="accH2")  # step i-2 accs
        nc.vector.memset(accH1[:], 0.0)
        nc.vector.memset(accH2[:], 0.0)

        ones14 = sing.tile([1, 4], f32, name="ones14")
        nc.vector.memset(ones14[:], 1.0)

        # quad-lane gravity workspace (3-partition replicated rows;
        # stage k occupies lanes 16k..16k+15)
        Wk = sing.tile([3, 64], f32, name="Wk")
        for _b in range(4):
            nc.vector.memset(Wk[0:3, 16 * _b:16 * _b + 1], 1.0)

        V = nc.vector

        def mlp_batched(SPp, SPv, tag):
            psH1 = psum.tile([32, 4], f32, tag="mlp", name="psH1" + tag)
            nc.tensor.matmul(psH1[:], sb["w1spt"][:], SPp[:], start=True,
                             stop=False)
            nc.tensor.matmul(psH1[:], sb["w1svt"][:], SPv[:], start=False,
                             stop=True)
            h1 = work.tile([32, 4], f32, tag="h1", name="h1" + tag)
            nc.scalar.activation(h1[:], psH1[:], Act.Tanh, bias=sb["b1t"][:])
            psH2 = psum.tile([32, 4], f32, tag="mlp", name="psH2" + tag)
            nc.tensor.matmul(psH2[:], sb["w2t"][:], h1[:], start=True, stop=True)
            h2t = work.tile([32, 4], f32, tag="h2", name="h2" + tag)
            nc.scalar.activation(h2t[:], psH2[:], Act.Tanh, bias=sb["b2t"][:])
            psNN = psum.tile([3, 4], f32, tag="mlp", name="psNN" + tag)
            nc.tensor.matmul(psNN[:], sb["w3et"][:], h2t[:], start=True,
                             stop=False)
            nc.tensor.matmul(psNN[:], sb["b3t"][:], ones14[:], start=False,
                             stop=True)
            return psNN

        def quad_chain(psF, tag):
            WR = Wk[0:3, 0:64].rearrange("p (s l) -> p s l", s=4)

            def P(off, n=1, step=1):
                return WR[0:3, 0:4, ds(off, n, step)]

            psFR = psF[0:3, 0:64].rearrange("p (s l) -> p s l", s=4)
            V.tensor_copy(P(13, 3), psFR[0:3, 0:4, 0:3])
            r2 = P(13)
            V.scalar_tensor_tensor(P(10), r2, C1Q_RSQ, r2, Alu.add, Alu.mult)
            V.scalar_tensor_tensor(P(11), P(10), C2_RSQ,
                                   sb["c0t"][0:3, 0:1].to_broadcast((3, 4, 1)),
                                   Alu.mult, Alu.add)
            V.tensor_tensor(P(12), P(11), P(11), Alu.mult)
            V.scalar_tensor_tensor(P(8), P(12), -0.5, r2, Alu.mult, Alu.mult)
            V.scalar_tensor_tensor(P(9), P(8), 1.5, P(11), Alu.add, Alu.mult)
            V.tensor_tensor(P(12), P(9), P(9), Alu.mult)
            V.scalar_tensor_tensor(P(8), P(12), -0.5, r2, Alu.mult, Alu.mult)
            V.scalar_tensor_tensor(P(9), P(8), 1.5, P(9), Alu.add, Alu.mult)
            V.tensor_tensor(P(1), P(9), P(9), Alu.mult)
            u_b2 = P(1).to_broadcast((3, 4, 2))
            V.tensor_tensor(P(5, 2, 2), P(14, 2), u_b2, Alu.mult)
            V.tensor_tensor(P(2, 2), P(1, 2, 6), u_b2, Alu.mult)
            V.tensor_tensor(P(6), P(1), P(9), Alu.mult)
            V.tensor_tensor(P(4), P(2), P(7), Alu.mult)

        def quad_head(SPp, tag):
            """4 stage positions in SPp cols -> psF (3,64) replicated rows."""
            sq4 = work.tile([3, 4], f32, tag="sq4", name="sq4" + tag)
            V.tensor_tensor(sq4[:], SPp[:], SPp[:], Alu.mult)
            psF = psum.tile([3, 64], f32, tag="gp", name="psF" + tag)
            for k in range(4):
                nc.tensor.matmul(psF[0:3, 16 * k:16 * k + 2],
                                 sq4[0:3, k:k + 1].to_broadcast((3, 3)),
                                 sb["cmv"][0:3, 0:2], start=True, stop=True)
                nc.tensor.matmul(psF[0:3, 16 * k + 2:16 * k + 3],
                                 SPp[0:3, k:k + 1].to_broadcast((3, 3)),
                                 sb["cmv"][0:3, 2:3], start=True, stop=True)
            return psF

        def gtail_quad(SPp, psNN, tag):
            """lanes -> Svec (3,4); accs for stages 1,2 (exact positions)."""
            pip = work.tile([3, 28], f32, tag="pip", name="pip" + tag)
            pipR = pip[0:3, 0:28].rearrange("p (s l) -> p s l", s=4)
            for k in range(4):
                V.tensor_scalar(pip[0:3, 7 * k:7 * k + 5],
                                Wk[0:3, 16 * k:16 * k + 5],
                                SPp[0:3, k:k + 1], None, Alu.mult)
            WkR = Wk[0:3, 0:64].rearrange("p (s l) -> p s l", s=4)
            nc.scalar.copy(pipR[0:3, 0:4, 5:7], WkR[0:3, 0:4, 1:3])
            Dt = work.tile([3, 28], f32, tag="Dt", name="Dt" + tag)
            for k in range(4):
                V.scalar_tensor_tensor(Dt[0:3, 7 * k:7 * k + 7], sb["g1c"][:],
                                       Wk[0:3, 16 * k + 5:16 * k + 6],
                                       sb["g0c"][:], Alu.mult, Alu.add)
            scr = work.tile([3, 28], f32, tag="scr", name="scr" + tag)
            V.tensor_tensor(scr[:], pip[:], Dt[:], Alu.mult)
            Svec = work.tile([3, 4], f32, tag="Svec", name="Svec" + tag)
            V.tensor_reduce(Svec[:], scr[0:3, 0:28].rearrange(
                "p (s l) -> p s l", s=4), axis=mybir.AxisListType.X, op=Alu.add)
            for k in (0, 1):
                V.scalar_tensor_tensor(accH1[0:3, k:k + 1], Svec[0:3, k:k + 1],
                                       Wk[0:3, 16 * k + 6:16 * k + 7],
                                       psNN[0:3, k:k + 1], Alu.mult, Alu.add)
            return Svec

        def grad_correct(Svec, SPp, psNN, ahT, k, cA, cB, tag):
            """stage k (2 or 3): a_k = (Svec_k + G-correction)*r3_k + nn_k.
            Delta folded: t = a_prev - ahat_prev; correction =
            cA*(p.t)*u*p + cB*t with cA=3*MU*h^2/x, cB=-MU*h^2/x."""
            kp = k - 2  # acc/ahT column supplying the delta (0 for k=2, 1 for k=3)
            t3 = work.tile([3, 1], f32, tag="t3", name="t3" + tag)
            V.tensor_tensor(t3[:], accH1[0:3, kp:kp + 1], ahT[0:3, kp:kp + 1],
                            Alu.subtract)
            psD = psum.tile([3, 1], f32, tag="mlp", name="psD" + tag)
            nc.tensor.matmul(psD[:], t3[0:3, 0:1].to_broadcast((3, 3)),
                             SPp[0:3, k:k + 1], start=True, stop=True)
            s2 = work.tile([3, 1], f32, tag="s2", name="s2" + tag)
            V.scalar_tensor_tensor(s2[:], psD[0:3, 0:1], cA,
                                   Wk[0:3, 16 * k + 1:16 * k + 2],
                                   Alu.mult, Alu.mult)
            mD = work.tile([3, 1], f32, tag="mD", name="mD" + tag)
            V.tensor_scalar(mD[:], t3[:], cB, None, Alu.mult)
            SD = work.tile([3, 1], f32, tag="SD", name="SD" + tag)
            V.scalar_tensor_tensor(SD[:], SPp[0:3, k:k + 1], s2[0:3, 0:1],
                                   mD[:], Alu.mult, Alu.add)
            Stot = work.tile([3, 1], f32, tag="Stot", name="Stot" + tag)
            V.tensor_tensor(Stot[:], Svec[0:3, k:k + 1], SD[:], Alu.add)
            V.scalar_tensor_tensor(accH1[0:3, k:k + 1], Stot[:],
                                   Wk[0:3, 16 * k + 6:16 * k + 7],
                                   psNN[0:3, k:k + 1], Alu.mult, Alu.add)

        def step(j, iv):
            t = f"_{j}"
            jp = (j - 1) % U
            posC = stP[0:3, jp:jp + 1]
            velC = stV[0:3, jp:jp + 1]
            # ---- predictor (DVE) ----
            G = nc.vector
            ahT = work.tile([3, 3], f32, tag="ahT", name="ahT" + t)
            G.scalar_tensor_tensor(ahT[:], accH1[0:3, 0:3], 2.0,
                                   accH2[0:3, 0:3], Alu.mult, Alu.subtract)
            G.tensor_copy(accH2[:], accH1[:])
            SPp = work.tile([3, 4], f32, tag="SPp", name="SPp" + t)
            SPv = work.tile([3, 4], f32, tag="SPv", name="SPv" + t)
            p4b = work.tile([3, 1], f32, tag="p4b", name="p4b" + t)
            V.scalar_tensor_tensor(SPp[0:3, 1:2], velC, h2, posC,
                                   Alu.mult, Alu.add)          # p2ex
            V.scalar_tensor_tensor(p4b[:], velC, hf, posC, Alu.mult, Alu.add)
            G.tensor_copy(SPp[0:3, 0:1], posC)
            G.scalar_tensor_tensor(SPp[0:3, 2:3], ahT[0:3, 0:1], h24,
                                   SPp[0:3, 1:2], Alu.mult, Alu.add)
            G.scalar_tensor_tensor(SPp[0:3, 3:4], ahT[0:3, 1:2], h22,
                                   p4b[:], Alu.mult, Alu.add)
            G.tensor_copy(SPv[0:3, 0:1], velC)
            G.scalar_tensor_tensor(SPv[0:3, 1:2], ahT[0:3, 0:1], h2, velC,
                                   Alu.mult, Alu.add)
            G.scalar_tensor_tensor(SPv[0:3, 2:3], ahT[0:3, 1:2], h2, velC,
                                   Alu.mult, Alu.add)
            G.scalar_tensor_tensor(SPv[0:3, 3:4], ahT[0:3, 2:3], hf, velC,
                                   Alu.mult, Alu.add)
            # ---- batched MLP (PE/ACT, concurrent) ----
            psNN = mlp_batched(SPp, SPv, t)
            # ---- 4-wide gravity at (exact, exact, predicted, predicted) ----
            psF = quad_head(SPp, t)
            quad_chain(psF, t)
            Svec = gtail_quad(SPp, psNN, t)          # also emits acc1, acc2
            # ---- gradient-corrected stages 3, 4 ----
            grad_correct(Svec, SPp, psNN, ahT, 2, C3A, C3B, "c3" + t)
            grad_correct(Svec, SPp, psNN, ahT, 3, C4A, C4B, "c4" + t)
            # ---- commit (one op after a4) ----
            t12 = work.tile([3, 1], f32, tag="t12", name="t12" + t)
            V.tensor_tensor(t12[:], accH1[0:3, 0:1], accH1[0:3, 1:2], Alu.add)
            s12 = work.tile([3, 1], f32, tag="s12", name="s12" + t)
            V.scalar_tensor_tensor(s12[:], accH1[0:3, 1:2], 2.0,
                                   accH1[0:3, 0:1], Alu.mult, Alu.add)
            t123 = work.tile([3, 1], f32, tag="t123", name="t123" + t)
            V.tensor_tensor(t123[:], t12[:], accH1[0:3, 2:3], Alu.add)
            V.scalar_tensor_tensor(stP[0:3, j:j + 1], t123[:], h26, p4b[:],
                                   Alu.mult, Alu.add)
            s123 = work.tile([3, 1], f32, tag="s123", name="s123" + t)
            V.scalar_tensor_tensor(s123[:], accH1[0:3, 2:3], 2.0, s12[:],
                                   Alu.mult, Alu.add)
            vpre = work.tile([3, 1], f32, tag="vpre", name="vpre" + t)
            V.scalar_tensor_tensor(vpre[:], s123[:], h6, velC,
                                   Alu.mult, Alu.add)
            V.scalar_tensor_tensor(stV[0:3, j:j + 1], accH1[0:3, 3:4], h6,
                                   vpre[:], Alu.mult, Alu.add)

        with tc.For_i(0, M, U) as iv:
            for j in range(U):
                step(j, iv)
                if j == H - 1 or j == U - 1:
                    lo = 0 if j == H - 1 else H
                    nc.sync.dma_start(
                        out_t[ds(iv + lo, H), 0:3].rearrange("r c -> c r"),
                        stP[0:3, lo:lo + H])
                    nc.sync.dma_start(
                        out_t[ds(iv + lo, H), 3:6].rearrange("r c -> c r"),
                        stV[0:3, lo:lo + H])

    nc.compile()
    return nc, names


_CACHE = {}


def _get_built(M, U=8, dt=60.0):
    key = (M, U, float(dt))
    if key not in _CACHE:
        _CACHE[key] = build(M, U, dt)
    return _CACHE[key]


def kernel(state0, eval_times, W1, b1, W2, b2, W3, b3, log_scale, t0, dt,
           **extra):
    M = int(np.asarray(eval_times).shape[0])
    U = 16 if M % 16 == 0 else (8 if M % 8 == 0 else 2)
    nc, names = _get_built(M, U, float(np.asarray(dt)))
    hd = make_host_inputs(np.asarray(state0), np.asarray(W1), np.asarray(b1),
                          np.asarray(W2), np.asarray(b2), np.asarray(W3),
                          np.asarray(b3), np.asarray(log_scale),
                          float(np.asarray(dt)))
    in_map = {names[k]: np.ascontiguousarray(v) for k, v in hd.items()}
    from concourse import bass_utils
    res = bass_utils.run_bass_kernel_spmd(
        nc, [dict(in_map) for _ in range(8)], core_ids=list(range(8)))
    out = res.results[0][names["out"]]
    return np.asarray(out, dtype=np.float32)


if __name__ == "__main__":
    # quick CoreSim functional check on a short horizon
    M = 8
    nc, names = build(M=M, U=M)
    import jax
    jax.config.update('jax_platforms', 'cpu')
    import reference as Rf
    inputs = Rf.setup_inputs()
    hd = make_host_inputs(np.asarray(inputs['state0']), np.asarray(inputs['W1']),
                          np.asarray(inputs['b1']), np.asarray(inputs['W2']),
                          np.asarray(inputs['b2']), np.asarray(inputs['W3']),
                          np.asarray(inputs['b3']),
                          np.asarray(inputs['log_scale']), float(inputs['dt']))
    from concourse.bass_interp import CoreSim
    sim = CoreSim(nc)
    for k, v in hd.items():
        sim.tensor(names[k])[:] = v
    sim.simulate()
    got = np.array(sim.tensor(names["out"]))
    exp = np.asarray(Rf.reference(**inputs))[:M]
    err = np.abs(got - exp)
    rel = np.linalg.norm(got - exp) / np.linalg.norm(exp)
    print("sim out[0]:", got[0])
    print("ref    [0]:", exp[0])
    print("sim out[-1]:", got[-1])
    print("ref     [-1]:", exp[-1])
    print(f"fro rel err over {M} steps: {rel:.3e}  absmax {err.max():.3e}")
    print("sim predicted time (ns):", sim.time)


# revision 19
# speedup vs baseline: 1.5413x; 1.1050x over previous
"""Trainium2 Bass kernel for nn_NeuralODE (RK4 satellite propagation, M=10000 steps).

Self-contained: takes FULL inputs (as produced by the problem's setup_inputs),
runs the sequential RK4 integration on NeuronCore 0 (replicated across cores
0-7 via SPMD), returns the full (M, 6) trajectory.

Design notes:
  - state (6,1) on partitions 0-5. MLP (6->32->32->3 tanh) on TensorE+ScalarE.
  - gravity J2..J5 evaluated via a factored form:
      g = r3inv * ( pos * S + e_z * R )
      S = -MU + u*D2(m) + u^2*D4(m) + (z*u^2)*D3(m) + (z*u^3)*D5(m)
      (m = z^2*u, u = 1/r^2; D_k linear in m -- the dropped m^2 terms
       contribute < 1e-9 relative)
    computed as a 5-lane tensor_tensor_reduce with per-partition (x/y/z)
    coefficient tables; R folded into two extra lanes of the z row.
  - 1/r via quadratic polynomial init (fitted on r^2 in [3.9e7, 5.1e7]) plus
    two Newton iterations (validated to sit within the f32 reference's own
    rounding envelope).
  - RK4 stage states & next-state built as tiny PE matmul accumulations:
      s2 = [p0 + (h/2)v0 ; v0 + (h/2)a1]
      s3 = [p2 + (h^2/4)a1 ; v0 + (h/2)a2]
      s4 = [p0 + h v0 + (h^2/2)a2 ; v0 + h a3]
      s' = [p0 + h v0 + (h^2/6)(a1+a2+a3) ; v0 + (h/6)(a1+2a2+2a3+a4)]
  - all dt-dependent constants are folded on the host into input tensors, so
    one compiled NEFF serves any dt.
"""
import os
import numpy as np
from contextlib import ExitStack

MU = 398600.4418
RE = 6378.137
J2 = 1.08262668e-3
J3 = -2.53265648e-6
J4 = -1.61962159e-6
J5 = -2.27296082e-7
R_REF = 7000.0
V_REF = 7.5
A_REF = V_REF * V_REF / R_REF

F32 = np.float32


# ----------------------------------------------------------------------------
# host-side constant folding
# ----------------------------------------------------------------------------

def _fit_rsqrt_quad(lo=3.9e7, hi=5.1e7):
    """least-squares quadratic fit of x^{-1/2} on [lo, hi] (Chebyshev nodes)."""
    xs = (np.cos(np.pi * (np.arange(4000) + 0.5) / 4000) + 1) / 2 * (hi - lo) + lo
    f = xs ** -0.5
    t = 2 * (xs - lo) / (hi - lo) - 1
    V = np.polynomial.chebyshev.chebvander(t, 2)
    c, *_ = np.linalg.lstsq(V, f, rcond=None)
    pt = np.polynomial.chebyshev.cheb2poly(c)
    a = 2 / (hi - lo)
    b = -1 - 2 * lo / (hi - lo)
    px = np.zeros(3)
    for k, ck in enumerate(pt):
        poly = np.array([1.0])
        for _ in range(k):
            poly = np.convolve(poly, np.array([b, a]))
        px[:len(poly)] += ck * poly
    return px  # [c0, c1, c2]: c0 + c1*x + c2*x^2


_PX = _fit_rsqrt_quad()
C0_RSQ = float(_PX[0])
C1_RSQ = float(_PX[1])
C2_RSQ = float(_PX[2])
C1Q_RSQ = C1_RSQ / C2_RSQ  # v0 = C2*((r2 + C1Q)*r2) + C0


def _gravity_tables():
    """(G1, G0) each (3, 7): D = G1*m + G0 per lane; lanes
    [const(-MU), u*D2, u2*D4, (u*zu)*D3, (u2*zu)*D5, R1(u), R2(u2)]."""
    G0 = np.zeros((3, 7))
    G1 = np.zeros((3, 7))
    G0[:, 0] = -MU
    c2 = J2 * MU * RE**2
    G0[:2, 1] = -1.5 * c2
    G1[:2, 1] = 7.5 * c2
    G0[2, 1] = -4.5 * c2
    G1[2, 1] = 7.5 * c2
    c4 = (15.0 / 8.0) * J4 * MU * RE**4
    G0[:2, 2] = c4
    G1[:2, 2] = -14.0 * c4
    G0[2, 2] = 5.0 * c4
    G1[2, 2] = -(70.0 / 3.0) * c4
    c3 = -2.5 * J3 * MU * RE**3
    G0[:2, 3] = 3.0 * c3
    G1[:2, 3] = -7.0 * c3
    G0[2, 3] = 6.0 * c3
    G1[2, 3] = -7.0 * c3
    c5 = (3.0 / 8.0) * J5 * MU * RE**5
    G0[:2, 4] = 35.0 * c5
    G1[:2, 4] = -210.0 * c5
    G0[2, 4] = 105.0 * c5
    G1[2, 4] = -315.0 * c5
    # R lanes (z row only): R = rho1*u + rho2*u2
    G0[2, 5] = 1.5 * J3 * MU * RE**3
    G0[2, 6] = -1.875 * J5 * MU * RE**5
    return F32(G1), F32(G0)


def _amat(alpha):
    """lhsT for out = [[I, alpha*I],[0, I]] @ s, split into pos/vel halves."""
    A = np.zeros((6, 6))
    A[:3, :3] = np.eye(3)
    A[:3, 3:] = alpha * np.eye(3)
    A[3:, 3:] = np.eye(3)
    Ap = A.copy(); Ap[3:, :] = 0.0           # pos rows only
    Av = A.copy(); Av[:3, :] = 0.0           # vel rows only
    return F32(Ap.T), F32(Av.T), F32(A.T)


def _bmat(alpha, beta):
    """lhsT (3,6) for out(6,1) = [alpha*a ; beta*a], a (3,1)."""
    B = np.zeros((6, 3))
    B[:3, :] = alpha * np.eye(3)
    B[3:, :] = beta * np.eye(3)
    return F32(B.T)


def make_host_inputs(state0, W1, b1, W2, b2, W3, b3, log_scale, dt):
    h = float(dt)
    scale_vec = np.array([1 / R_REF] * 3 + [1 / V_REF] * 3, np.float64)
    W1s = F32(np.float64(W1) * scale_vec[None, :])
    amp = F32(F32(np.exp(F32(log_scale))) * F32(A_REF))
    W3e = F32(np.float64(W3) * np.float64(amp))
    b3e = F32(np.float64(b3) * np.float64(amp))
    G1, G0 = _gravity_tables()
    d = {
        "w1spt": np.ascontiguousarray(W1s[:, 0:3].T),   # (3,32)
        "w1svt": np.ascontiguousarray(W1s[:, 3:6].T),   # (3,32)
        "w2t": np.ascontiguousarray(F32(W2).T),         # (32,32)
        "w3et": np.ascontiguousarray(W3e.T),            # (32,3)
        "b1t": F32(b1).reshape(32, 1),
        "b2t": F32(b2).reshape(32, 1),
        "b3t": b3e.reshape(1, 3),
        "g1c": G1, "g0c": G0,                           # (3,7)
        "c0t": F32([[C0_RSQ]] * 3),                     # (3,1)
        "cmv": F32([[1, 0, 0], [1, 0, 0], [1, 1, 1]]),  # (3,3)
        "ones13": F32([[1, 1, 1]]),                     # (1,3)
        "s0t": F32(state0).reshape(6, 1),
    }
    return d


# ----------------------------------------------------------------------------
# kernel builder
# ----------------------------------------------------------------------------

def build(M=10000, U=8, dt=60.0):
    assert U % 2 == 0 and M % U == 0
    return _build_inner(M, U, U // 2, float(dt))


def _build_inner(M, U, H, h):
    import concourse.bacc as bacc
    import concourse.tile as tile
    from concourse import mybir
    from concourse.bass import ds

    f32 = mybir.dt.float32
    Alu = mybir.AluOpType
    Act = mybir.ActivationFunctionType

    hf = float(F32(h))
    h2 = float(F32(hf / 2))
    h6 = float(F32(hf / 6))
    hh = float(F32(F32(hf) * F32(hf)))
    h24 = float(F32(hh / 4))
    h22 = float(F32(hh / 2))
    h26 = float(F32(hh / 6))
    C3A = float(F32(3.0 * MU * h24))
    C3B = float(F32(-MU * h24))
    C4A = float(F32(3.0 * MU * h22))
    C4B = float(F32(-MU * h22))

    nc = bacc.Bacc(None, target_bir_lowering=False, debug=False)
    names = {}

    with tile.TileContext(nc) as tc, ExitStack() as ctx:
        dram = ctx.enter_context(tc.tile_pool(name="dram", bufs=1, space="DRAM"))
        sing = ctx.enter_context(tc.tile_pool(name="sing", bufs=1))
        work = ctx.enter_context(tc.tile_pool(name="work", bufs=2))
        psum = ctx.enter_context(tc.tile_pool(name="ps", bufs=2, space="PSUM"))

        shapes = {
            "w1spt": (3, 32), "w1svt": (3, 32), "w2t": (32, 32),
            "w3et": (32, 3), "b1t": (32, 1), "b2t": (32, 1), "b3t": (1, 3),
            "g1c": (3, 7), "g0c": (3, 7), "c0t": (3, 1), "cmv": (3, 3),
            "ones13": (1, 3), "s0t": (6, 1),
        }
        dtiles = {}
        sb = {}
        for nm, shp in shapes.items():
            dtiles[nm] = dram.tile(list(shp), f32, kind="ExternalInput", name=nm,
                                   uniquify=False)
            names[nm] = nm
            sb[nm] = sing.tile(list(shp), f32, name="sb_" + nm)
            nc.sync.dma_start(sb[nm][:], dtiles[nm][:])

        out_t = dram.tile([M, 6], f32, kind="ExternalOutput", name="out",
                          uniquify=False)
        names["out"] = "out"

        # staging: positions and velocities, one column per step in the body
        stP = sing.tile([3, U], f32, name="stP")
        stV = sing.tile([3, U], f32, name="stV")
        nc.sync.dma_start(stP[0:3, U - 1:U], dtiles["s0t"][0:3, 0:1])
        nc.sync.dma_start(stV[0:3, U - 1:U], dtiles["s0t"][3:6, 0:1])

        # acceleration history for the stage-state predictor
        accH1 = sing.tile([3, 4], f32, name="accH1")  # step i-1 accs
        accH2 = sing.tile([3, 4], f32, name="accH2")  # step i-2 accs
        nc.vector.memset(accH1[:], 0.0)
        nc.vector.memset(accH2[:], 0.0)

        ones14 = sing.tile([1, 4], f32, name="ones14")
        nc.vector.memset(ones14[:], 1.0)

        # quad-lane gravity workspace (3-partition replicated rows;
        # stage k occupies lanes 16k..16k+15)
        Wk = sing.tile([3, 64], f32, name="Wk")
        for _b in range(4):
            nc.vector.memset(Wk[0:3, 16 * _b:16 * _b + 1], 1.0)

        V = nc.vector

        def mlp_batched(SPp, SPv, tag):
            psH1 = psum.tile([32, 4], f32, tag="mlp", name="psH1" + tag)
            nc.tensor.matmul(psH1[:], sb["w1spt"][:], SPp[:], start=True,
                             stop=False)
            nc.tensor.matmul(psH1[:], sb["w1svt"][:], SPv[:], start=False,
                             stop=True)
            h1 = work.tile([32, 4], f32, tag="h1", name="h1" + tag)
            nc.scalar.activation(h1[:], psH1[:], Act.Tanh, bias=sb["b1t"][:])
            psH2 = psum.tile([32, 4], f32, tag="mlp", name="psH2" + tag)
            nc.tensor.matmul(psH2[:], sb["w2t"][:], h1[:], start=True, stop=True)
            h2t = work.tile([32, 4], f32, tag="h2", name="h2" + tag)
            nc.scalar.activation(h2t[:], psH2[:], Act.Tanh, bias=sb["b2t"][:])
            psNN = psum.tile([3, 4], f32, tag="mlp", name="psNN" + tag)
            nc.tensor.matmul(psNN[:], sb["w3et"][:], h2t[:], start=True,
                             stop=False)
            nc.tensor.matmul(psNN[:], sb["b3t"][:], ones14[:], start=False,
                             stop=True)
            return psNN

        def quad_chain(psF, tag):
            WR = Wk[0:3, 0:64].rearrange("p (s l) -> p s l", s=4)

            def P(off, n=1, step=1):
                return WR[0:3, 0:4, ds(off, n, step)]

            psFR = psF[0:3, 0:64].rearrange("p (s l) -> p s l", s=4)
            V.tensor_copy(P(13, 3), psFR[0:3, 0:4, 0:3])
            r2 = P(13)
            V.scalar_tensor_tensor(P(10), r2, C1Q_RSQ, r2, Alu.add, Alu.mult)
            V.scalar_tensor_tensor(P(11), P(10), C2_RSQ,
                                   sb["c0t"][0:3, 0:1].to_broadcast((3, 4, 1)),
                                   Alu.mult, Alu.add)
            V.tensor_tensor(P(12), P(11), P(11), Alu.mult)
            V.scalar_tensor_tensor(P(8), P(12), -0.5, r2, Alu.mult, Alu.mult)
            V.scalar_tensor_tensor(P(9), P(8), 1.5, P(11), Alu.add, Alu.mult)
            V.tensor_tensor(P(12), P(9), P(9), Alu.mult)
            V.scalar_tensor_tensor(P(8), P(12), -0.5, r2, Alu.mult, Alu.mult)
            V.scalar_tensor_tensor(P(9), P(8), 1.5, P(9), Alu.add, Alu.mult)
            V.tensor_tensor(P(1), P(9), P(9), Alu.mult)
            u_b2 = P(1).to_broadcast((3, 4, 2))
            V.tensor_tensor(P(5, 2, 2), P(14, 2), u_b2, Alu.mult)
            V.tensor_tensor(P(2, 2), P(1, 2, 6), u_b2, Alu.mult)
            V.tensor_tensor(P(6), P(1), P(9), Alu.mult)
            V.tensor_tensor(P(4), P(2), P(7), Alu.mult)

        def quad_head(SPp, tag):
            """4 stage positions in SPp cols -> psF (3,64) replicated rows."""
            sq4 = work.tile([3, 4], f32, tag="sq4", name="sq4" + tag)
            V.tensor_tensor(sq4[:], SPp[:], SPp[:], Alu.mult)
            psF = psum.tile([3, 64], f32, tag="gp", name="psF" + tag)
            for k in range(4):
                nc.tensor.matmul(psF[0:3, 16 * k:16 * k + 2],
                                 sq4[0:3, k:k + 1].to_broadcast((3, 3)),
                                 sb["cmv"][0:3, 0:2], start=True, stop=True)
                nc.tensor.matmul(psF[0:3, 16 * k + 2:16 * k + 3],
                                 SPp[0:3, k:k + 1].to_broadcast((3, 3)),
                                 sb["cmv"][0:3, 2:3], start=True, stop=True)
            return psF

        def gtail_quad(SPp, psNN, tag):
            """lanes -> Svec (3,4); accs for stages 1,2 (exact positions).
            All per-stage multiplies done as single (3,4,.) broadcast ops."""
            pip = work.tile([3, 28], f32, tag="pip", name="pip" + tag)
            pipR = pip[0:3, 0:28].rearrange("p (s l) -> p s l", s=4)
            WkR = Wk[0:3, 0:64].rearrange("p (s l) -> p s l", s=4)
            pos_b = SPp[0:3, 0:4].rearrange("p (s l) -> p s l", l=1
                                            ).to_broadcast((3, 4, 5))
            V.tensor_tensor(pipR[0:3, 0:4, 0:5], WkR[0:3, 0:4, 0:5], pos_b,
                            Alu.mult)
            nc.scalar.copy(pipR[0:3, 0:4, 5:7], WkR[0:3, 0:4, 1:3])
            Dt = work.tile([3, 28], f32, tag="Dt", name="Dt" + tag)
            DtR = Dt[0:3, 0:28].rearrange("p (s l) -> p s l", s=4)
            m_b = WkR[0:3, 0:4, 5:6].to_broadcast((3, 4, 7))
            g1_b = sb["g1c"][0:3, 0:7].rearrange("p (s l) -> p s l", s=1
                                                 ).to_broadcast((3, 4, 7))
            g0_b = sb["g0c"][0:3, 0:7].rearrange("p (s l) -> p s l", s=1
                                                 ).to_broadcast((3, 4, 7))
            V.tensor_tensor(DtR[:], g1_b, m_b, Alu.mult)
            V.tensor_tensor(DtR[:], DtR[:], g0_b, Alu.add)
            scr = work.tile([3, 28], f32, tag="scr", name="scr" + tag)
            V.tensor_tensor(scr[:], pip[:], Dt[:], Alu.mult)
            Svec = work.tile([3, 4], f32, tag="Svec", name="Svec" + tag)
            V.tensor_reduce(Svec[:], scr[0:3, 0:28].rearrange(
                "p (s l) -> p s l", s=4), axis=mybir.AxisListType.X, op=Alu.add)
            for k in (0, 1):
                V.scalar_tensor_tensor(accH1[0:3, k:k + 1], Svec[0:3, k:k + 1],
                                       Wk[0:3, 16 * k + 6:16 * k + 7],
                                       psNN[0:3, k:k + 1], Alu.mult, Alu.add)
            return Svec

        def grad_correct(Svec, SPp, psNN, ahT, k, cA, cB, tag):
            """stage k (2 or 3): a_k = (Svec_k + G-correction)*r3_k + nn_k.
            Delta folded: t = a_prev - ahat_prev; correction =
            cA*(p.t)*u*p + cB*t with cA=3*MU*h^2/x, cB=-MU*h^2/x."""
            kp = k - 2  # acc/ahT column supplying the delta (0 for k=2, 1 for k=3)
            t3 = work.tile([3, 1], f32, tag="t3", name="t3" + tag)
            V.tensor_tensor(t3[:], accH1[0:3, kp:kp + 1], ahT[0:3, kp:kp + 1],
                            Alu.subtract)
            psD = psum.tile([3, 1], f32, tag="mlp", name="psD" + tag)
            nc.tensor.matmul(psD[:], t3[0:3, 0:1].to_broadcast((3, 3)),
                             SPp[0:3, k:k + 1], start=True, stop=True)
            s2 = work.tile([3, 1], f32, tag="s2", name="s2" + tag)
            V.scalar_tensor_tensor(s2[:], psD[0:3, 0:1], cA,
                                   Wk[0:3, 16 * k + 1:16 * k + 2],
                                   Alu.mult, Alu.mult)
            mD = work.tile([3, 1], f32, tag="mD", name="mD" + tag)
            V.tensor_scalar(mD[:], t3[:], cB, None, Alu.mult)
            SD = work.tile([3, 1], f32, tag="SD", name="SD" + tag)
            V.scalar_tensor_tensor(SD[:], SPp[0:3, k:k + 1], s2[0:3, 0:1],
                                   mD[:], Alu.mult, Alu.add)
            Stot = work.tile([3, 1], f32, tag="Stot", name="Stot" + tag)
            V.tensor_tensor(Stot[:], Svec[0:3, k:k + 1], SD[:], Alu.add)
            V.scalar_tensor_tensor(accH1[0:3, k:k + 1], Stot[:],
                                   Wk[0:3, 16 * k + 6:16 * k + 7],
                                   psNN[0:3, k:k + 1], Alu.mult, Alu.add)

        def step(j, iv):
            t = f"_{j}"
            jp = (j - 1) % U
            posC = stP[0:3, jp:jp + 1]
            velC = stV[0:3, jp:jp + 1]
            # ---- predictor (DVE) ----
            G = nc.vector
            ahT = work.tile([3, 3], f32, tag="ahT", name="ahT" + t)
            G.scalar_tensor_tensor(ahT[:], accH1[0:3, 0:3], 2.0,
                                   accH2[0:3, 0:3], Alu.mult, Alu.subtract)
            G.tensor_copy(accH2[:], accH1[:])
            SPp = work.tile([3, 4], f32, tag="SPp", name="SPp" + t)
            SPv = work.tile([3, 4], f32, tag="SPv", name="SPv" + t)
            p4b = work.tile([3, 1], f32, tag="p4b", name="p4b" + t)
            V.scalar_tensor_tensor(SPp[0:3, 1:2], velC, h2, posC,
                                   Alu.mult, Alu.add)          # p2ex
            V.scalar_tensor_tensor(p4b[:], velC, hf, posC, Alu.mult, Alu.add)
            G.tensor_copy(SPp[0:3, 0:1], posC)
            G.scalar_tensor_tensor(SPp[0:3, 2:3], ahT[0:3, 0:1], h24,
                                   SPp[0:3, 1:2], Alu.mult, Alu.add)
            G.scalar_tensor_tensor(SPp[0:3, 3:4], ahT[0:3, 1:2], h22,
                                   p4b[:], Alu.mult, Alu.add)
            G.tensor_copy(SPv[0:3, 0:1], velC)
            G.scalar_tensor_tensor(SPv[0:3, 1:2], ahT[0:3, 0:1], h2, velC,
                                   Alu.mult, Alu.add)
            G.scalar_tensor_tensor(SPv[0:3, 2:3], ahT[0:3, 1:2], h2, velC,
                                   Alu.mult, Alu.add)
            G.scalar_tensor_tensor(SPv[0:3, 3:4], ahT[0:3, 2:3], hf, velC,
                                   Alu.mult, Alu.add)
            # ---- batched MLP (PE/ACT, concurrent) ----
            psNN = mlp_batched(SPp, SPv, t)
            # ---- 4-wide gravity at (exact, exact, predicted, predicted) ----
            psF = quad_head(SPp, t)
            quad_chain(psF, t)
            Svec = gtail_quad(SPp, psNN, t)          # also emits acc1, acc2
            # ---- gradient-corrected stages 3, 4 ----
            grad_correct(Svec, SPp, psNN, ahT, 2, C3A, C3B, "c3" + t)
            grad_correct(Svec, SPp, psNN, ahT, 3, C4A, C4B, "c4" + t)
            # ---- commit (one op after a4) ----
            t12 = work.tile([3, 1], f32, tag="t12", name="t12" + t)
            V.tensor_tensor(t12[:], accH1[0:3, 0:1], accH1[0:3, 1:2], Alu.add)
            s12 = work.tile([3, 1], f32, tag="s12", name="s12" + t)
            V.scalar_tensor_tensor(s12[:], accH1[0:3, 1:2], 2.0,
                                   accH1[0:3, 0:1], Alu.mult, Alu.add)
            t123 = work.tile([3, 1], f32, tag="t123", name="t123" + t)
            V.tensor_tensor(t123[:], t12[:], accH1[0:3, 2:3], Alu.add)
            V.scalar_tensor_tensor(stP[0:3, j:j + 1], t123[:], h26, p4b[:],
                                   Alu.mult, Alu.add)
            s123 = work.tile([3, 1], f32, tag="s123", name="s123" + t)
            V.scalar_tensor_tensor(s123[:], accH1[0:3, 2:3], 2.0, s12[:],
                                   Alu.mult, Alu.add)
            vpre = work.tile([3, 1], f32, tag="vpre", name="vpre" + t)
            V.scalar_tensor_tensor(vpre[:], s123[:], h6, velC,
                                   Alu.mult, Alu.add)
            V.scalar_tensor_tensor(stV[0:3, j:j + 1], accH1[0:3, 3:4], h6,
                                   vpre[:], Alu.mult, Alu.add)

        with tc.For_i(0, M, U) as iv:
            for j in range(U):
                step(j, iv)
                if j == H - 1 or j == U - 1:
                    lo = 0 if j == H - 1 else H
                    nc.sync.dma_start(
                        out_t[ds(iv + lo, H), 0:3].rearrange("r c -> c r"),
                        stP[0:3, lo:lo + H])
                    nc.sync.dma_start(
                        out_t[ds(iv + lo, H), 3:6].rearrange("r c -> c r"),
                        stV[0:3, lo:lo + H])

    nc.compile()
    return nc, names


_CACHE = {}


def _get_built(M, U=8, dt=60.0):
    key = (M, U, float(dt))
    if key not in _CACHE:
        _CACHE[key] = build(M, U, dt)
    return _CACHE[key]


def kernel(state0, eval_times, W1, b1, W2, b2, W3, b3, log_scale, t0, dt,
           **extra):
    M = int(np.asarray(eval_times).shape[0])
    U = 16 if M % 16 == 0 else (8 if M % 8 == 0 else 2)
    nc, names = _get_built(M, U, float(np.asarray(dt)))
    hd = make_host_inputs(np.asarray(state0), np.asarray(W1), np.asarray(b1),
                          np.asarray(W2), np.asarray(b2), np.asarray(W3),
                          np.asarray(b3), np.asarray(log_scale),
                          float(np.asarray(dt)))
    in_map = {names[k]: np.ascontiguousarray(v) for k, v in hd.items()}
    from concourse import bass_utils
    res = bass_utils.run_bass_kernel_spmd(
        nc, [dict(in_map) for _ in range(8)], core_ids=list(range(8)))
    out = res.results[0][names["out"]]
    return np.asarray(out, dtype=np.float32)


if __name__ == "__main__":
    # quick CoreSim functional check on a short horizon
    M = 8
    nc, names = build(M=M, U=M)
    import jax
    jax.config.update('jax_platforms', 'cpu')
    import reference as Rf
    inputs = Rf.setup_inputs()
    hd = make_host_inputs(np.asarray(inputs['state0']), np.asarray(inputs['W1']),
                          np.asarray(inputs['b1']), np.asarray(inputs['W2']),
                          np.asarray(inputs['b2']), np.asarray(inputs['W3']),
                          np.asarray(inputs['b3']),
                          np.asarray(inputs['log_scale']), float(inputs['dt']))
    from concourse.bass_interp import CoreSim
    sim = CoreSim(nc)
    for k, v in hd.items():
        sim.tensor(names[k])[:] = v
    sim.simulate()
    got = np.array(sim.tensor(names["out"]))
    exp = np.asarray(Rf.reference(**inputs))[:M]
    err = np.abs(got - exp)
    rel = np.linalg.norm(got - exp) / np.linalg.norm(exp)
    print("sim out[0]:", got[0])
    print("ref    [0]:", exp[0])
    print("sim out[-1]:", got[-1])
    print("ref     [-1]:", exp[-1])
    print(f"fro rel err over {M} steps: {rel:.3e}  absmax {err.max():.3e}")
    print("sim predicted time (ns):", sim.time)
